# revision 11
# baseline (speedup 1.0000x reference)
"""Trainium2 Bass kernel for a 2-layer categorical GIN encoder.

Graph: N=100000 nodes, E=1600000 edges, 256-dim features.

    x   = concat_i emb_i[x_cat[:, i]]                  # [N, 256]
    h1  = LN1(relu(relu((x + A x) @ w1a + b1a) @ w1b + b1b))
    out = LN2(relu((h1 + A h1) @ w2a + b2a) @ w2b + b2b)

where (A x)[d] = sum over edges (s -> d) of x[s].

Strategy (8 NeuronCores, SPMD):
  * Linearity trick: (x + Ax) @ w1a == z + A z with z = x @ w1a, and
    z[n] = sum_i t_i[x_cat[n, i]] where t_i = emb_i @ w1a[64i:64i+64, :].
    t tables are built on device (bf16); each core builds its z shard by
    dma_gather of t rows; shards are AllGathered in 4 row slices
    (overlapped with the z build) into a slice-major full table
    (global row = (slice, core, row)) so collective in/outs stay
    contiguous.
  * Edges are sorted by destination and sharded by destination range
    (12500 nodes per core) -> the aggregation needs no collective.
  * Per-edge gathers of z[src] use the InstDMAGatherAnt ucode whose
    ~1us SWDGE overhead is per *call*: calls cover up to 8 chunks
    (1024 edges, the SWDGE descriptor-ring cap) of a 4-tile group,
    one bucket (25000 permuted rows, int16 index limit) at a time,
    rotating over 4 SWDGE queues.  Edges are src-sorted inside each
    (tile, bucket) segment so descriptors read ascending addresses.
    Slots beyond a segment's max-over-cores edge count carry idx -1
    (skipped by the ucode, zero HBM traffic); slots between this
    core's count and the max carry idx 0 (row 0 fetch) so the valid
    count baked into the SPMD program is core-uniform.
  * Aggregation per tile: one-hot S[e, d] = (dst_rel[e] == d) built on
    DVE (bf16), agg += S.T @ G in PSUM (bf16 in, fp32 accumulate);
    padding slots have dst_rel -1 and select zero.
  * MLP per tile: PE transposes + bf16 weight matmuls; relu / LayerNorm
    application / PSUM->SBUF casts run on the Activation engine
    (func(scale*x+bias) with per-partition scale/bias) to unload DVE;
    bn_stats/bn_aggr remain on DVE.  Layer 1 also applies w2a so the
    second (also sliced) AllGather ships z2 = h1 @ w2a.
"""

import numpy as np

# ---------------------------------------------------------------------------
# Problem constants (hardcoded per contest contract)
# ---------------------------------------------------------------------------
N = 100000        # nodes
E = 1600000       # edges
D = 256           # feature dim (in = hidden = out)
EMB = 64          # per-field embedding dim
V = 1000          # categories per field
NF = 4            # categorical fields
NC = 8            # NeuronCores
P = 128           # partitions
LN_EPS = 1e-5

NSH = N // NC             # nodes per core (12500)
NT = (NSH + P - 1) // P   # node tiles per core (98)
LAST_VALID = NSH - (NT - 1) * P  # valid rows in last tile (84)

NBUK = 4                  # source buckets == allgather slices
GSZ = 4                   # dst tiles per gather group
CGT = 4                   # tiles per phase-C gather call
MAXC = 8                  # chunks per dma_gather call (1024-desc ring cap)
NSL = 4                   # allgather slices

# slice layout: tiles per slice -> local row ranges
_SL_TILES = [25, 25, 25, 23]
SL_ROWS = []              # (row_base, nrows) per slice
_rb = 0
for _s in range(NSL):
    _nr = min(_SL_TILES[_s] * P, NSH - _rb)
    SL_ROWS.append((_rb, _nr))
    _rb += _nr
assert _rb == NSH
GLOB_OFF = [0]
for _rb, _nr in SL_ROWS:
    GLOB_OFF.append(GLOB_OFF[-1] + _nr * NC)


def _permute_rows(node):
    """node id -> slice-major global row (slice, core, local row)."""
    k = node // NSH
    r = node - k * NSH
    out = np.zeros_like(node)
    for s, (rb, nr) in enumerate(SL_ROWS):
        m = (r >= rb) & (r < rb + nr)
        out = np.where(m, GLOB_OFF[s] + k * nr + (r - rb), out)
    return out


def _wrap_idx(flat):
    """[L] int16 (L % 16 == 0) -> [128, L//16] wrapped + replicated."""
    w = flat.reshape(-1, 16).T.copy()           # [16, L//16]
    return np.tile(w, (8, 1))                   # [128, L//16]


# ---------------------------------------------------------------------------
# Host-side preprocessing: shard + sort edges, build per-core metadata
# ---------------------------------------------------------------------------
def _prep_meta(x_cat, edge_index):
    import ml_dtypes

    src = np.asarray(edge_index[0], dtype=np.int64)
    dst = np.asarray(edge_index[1], dtype=np.int64)
    prow = _permute_rows(src)

    order = np.argsort(dst, kind="stable")
    dst_s = dst[order]
    prow_s = prow[order]
    bounds = np.searchsorted(dst_s, np.arange(NC + 1) * NSH)

    per_core = []
    counts_tb = np.zeros((NC, NT * NBUK), dtype=np.int64)
    for k in range(NC):
        lo, hi = bounds[k], bounds[k + 1]
        d_k = dst_s[lo:hi] - k * NSH
        p_k = prow_s[lo:hi]
        t_k = d_k // P
        rel = (d_k - t_k * P).astype(np.int64)
        b_k = np.searchsorted(GLOB_OFF, p_k, side="right") - 1
        key = t_k * NBUK + b_k
        # sort by (tile, bucket, src row) -> ascending gather addresses
        o2 = np.lexsort((p_k, key))
        key = key[o2]
        counts = np.bincount(key, minlength=NT * NBUK)
        counts_tb[k] = counts
        per_core.append((p_k[o2], rel[o2], key, counts))

    m_tb = counts_tb.max(axis=0)                       # [NT*NBUK] valid slots
    Cb = (-(-m_tb // P)).reshape(NT, NBUK)             # [NT, NBUK] chunks

    # global chunk column layout: group-major, bucket-major inside group
    n_groups = -(-NT // GSZ)
    col_start = np.zeros((NT, NBUK), dtype=np.int64)
    call_cols = []           # per group: [(b, col_lo, col_hi, n_valid)]
    cc = 0
    for g in range(n_groups):
        tiles = list(range(g * GSZ, min((g + 1) * GSZ, NT)))
        calls = []
        for b in range(NBUK):
            lo = cc
            for t in tiles:
                col_start[t, b] = cc
                cc += Cb[t, b]
            # split into <=MAXC-chunk calls (SWDGE ring cap).  nv =
            # num_idxs_reg = slots up to the last valid one: mid-call
            # padding gathers row 0 (mid-list -1 crashes the ucode);
            # only the call's tail may carry -1 (skipped).
            for s in range(lo, cc, MAXC):
                e = min(s + MAXC, cc)
                nv = 0
                for t in tiles:
                    a0, a1 = col_start[t, b], col_start[t, b] + Cb[t, b]
                    ov_lo, ov_hi = max(s, a0), min(e, a1)
                    if ov_hi <= ov_lo:
                        continue
                    mvv = int(m_tb[t * NBUK + b])
                    sl_lo = (ov_lo - a0) * P
                    sl_hi = (ov_hi - a0) * P
                    seg_valid = max(0, min(mvv, sl_hi) - sl_lo)
                    if seg_valid > 0:
                        nv = (ov_lo - s) * P + seg_valid
                if e > s:
                    calls.append((b, int(s), int(e), int(nv)))
        call_cols.append(calls)
    CT = int(cc)

    tile_ranges = [[(int(col_start[t, b]), int(col_start[t, b] + Cb[t, b]))
                    for b in range(NBUK) if Cb[t, b] > 0] for t in range(NT)]

    idx_meta, drel_meta = [], []
    for k in range(NC):
        p_k, rel_k, key_k, counts = per_core[k]
        starts = np.zeros(NT * NBUK, dtype=np.int64)
        np.cumsum(counts[:-1], out=starts[1:])
        j_seg = np.arange(len(p_k)) - starts[key_k]     # rank within (t,b)
        col = col_start.reshape(-1)[key_k] + j_seg // P
        slot = col * P + (j_seg % P)                    # global flat slot

        idxflat = np.full(CT * P, -1, dtype=np.int16)
        drelflat = np.full(CT * P, -1.0, dtype=np.float32)
        # padding below each call's valid cut fetches row 0 (idx 0);
        # everything after the cut stays -1 (skipped tail)
        for calls in call_cols:
            for b, c_lo, c_hi, nv in calls:
                base = c_lo * P
                idxflat[base:base + nv] = 0
        idxflat[slot] = (p_k - np.asarray(GLOB_OFF)[key_k % NBUK]).astype(np.int16)
        drelflat[slot] = rel_k

        idx_meta.append(_wrap_idx(idxflat))
        drel_meta.append(
            drelflat.reshape(CT, P).T.astype(ml_dtypes.bfloat16).copy())

    # phase-C index stream: slot (tile_in_call*NF + f)*128 + p
    x_cat = np.asarray(x_cat, dtype=np.int64)
    cmeta16 = []
    for k in range(NC):
        xc = x_cat[k * NSH:(k + 1) * NSH]               # [NSH, NF]
        rows = (xc + np.arange(NF)[None, :] * V).astype(np.int16)
        cm = np.zeros((NT, NF, P), dtype=np.int16)
        for t in range(NT):
            v = min(P, NSH - t * P)
            cm[t, :, :v] = rows[t * P:t * P + v].T
        cmeta16.append(_wrap_idx(cm.reshape(-1)))

    Cb_list = [[int(Cb[t, b]) for b in range(NBUK)] for t in range(NT)]
    return Cb_list, call_cols, tile_ranges, cmeta16, idx_meta, drel_meta


# ---------------------------------------------------------------------------
# Device program
# ---------------------------------------------------------------------------
_PROGRAM_CACHE = {}


def _build_program(Cb, call_cols, tile_ranges,
                   use_biases=False, use_ln_gb=False, num_cores=NC):
    """Build + compile the SPMD Bass program."""
    import concourse.bacc as bacc
    import concourse.bass as bass
    import concourse.tile as tile
    from concourse import mybir

    f32 = mybir.dt.float32
    i16 = mybir.dt.int16
    bf16 = mybir.dt.bfloat16
    AF = mybir.ActivationFunctionType

    nc = bacc.Bacc("TRN2", target_bir_lowering=False, debug=False,
                   num_devices=num_cores)

    CT = call_cols[-1][-1][2]
    n_groups = len(call_cols)
    NTC = -(-NT // CGT)
    Cg_max = max(calls[-1][2] - calls[0][1] for calls in call_cols)

    # ---- external I/O ----
    embT_d = nc.dram_tensor("embT", [NF, EMB, V], f32, kind="ExternalInput")
    w1a_d = nc.dram_tensor("w1a", [D, D], f32, kind="ExternalInput")
    w1b_d = nc.dram_tensor("w1b", [D, D], f32, kind="ExternalInput")
    w2a_d = nc.dram_tensor("w2a", [D, D], f32, kind="ExternalInput")
    w2b_d = nc.dram_tensor("w2b", [D, D], f32, kind="ExternalInput")
    cmeta_d = nc.dram_tensor("cmeta16", [128, NT * NF * P // 16], i16,
                             kind="ExternalInput")
    idx_d = nc.dram_tensor("idx_meta", [128, CT * P // 16], i16,
                           kind="ExternalInput")
    drel_d = nc.dram_tensor("drel_meta", [P, CT], bf16, kind="ExternalInput")
    iota_d = nc.dram_tensor("iota_row", [P, P], bf16, kind="ExternalInput")
    ident_d = nc.dram_tensor("identity", [P, P], f32, kind="ExternalInput")
    bias_d = None
    if use_biases or use_ln_gb:
        # rows: b1a, b1b, b2a, b2b, ln1_g, ln1_b, ln2_g, ln2_b
        bias_d = nc.dram_tensor("biasrows", [8, D], f32, kind="ExternalInput")
    out_d = nc.dram_tensor("out", [NSH, D], f32, kind="ExternalOutput")

    groups = [list(range(num_cores))]

    from contextlib import ExitStack

    with tile.TileContext(nc) as tc, ExitStack() as ctx:
        singles = ctx.enter_context(tc.tile_pool(name="singles", bufs=1))
        dram = ctx.enter_context(tc.tile_pool(name="dram", bufs=1, space="DRAM"))
        meta_p = ctx.enter_context(tc.tile_pool(name="meta", bufs=4))
        gath_p = ctx.enter_context(tc.tile_pool(name="gath", bufs=2))
        g4_p = ctx.enter_context(tc.tile_pool(name="g4", bufs=2))
        sel_p = ctx.enter_context(tc.tile_pool(name="sel", bufs=2))
        work_p = ctx.enter_context(tc.tile_pool(name="work", bufs=4))
        stat_p = ctx.enter_context(tc.tile_pool(name="stat", bufs=4))
        ps_agg = ctx.enter_context(tc.tile_pool(name="ps_agg", bufs=3, space="PSUM"))
        ps_tr = ctx.enter_context(tc.tile_pool(name="ps_tr", bufs=2, space="PSUM"))
        ps_mm = ctx.enter_context(tc.tile_pool(name="ps_mm", bufs=2, space="PSUM"))

        # ---- internal DRAM tables ----
        t_dram = dram.tile([NF * V, D], bf16)
        z_shard = dram.tile([NSH, D], bf16)
        z2_shard = dram.tile([NSH, D], bf16)
        z_full = [dram.tile([nr * num_cores, D], bf16, addr_space="Shared",
                            name=f"z_full_{s}")
                  for s, (rb, nr) in enumerate(SL_ROWS)]
        z2_full = [dram.tile([nr * num_cores, D], bf16, addr_space="Shared",
                             name=f"z2_full_{s}")
                   for s, (rb, nr) in enumerate(SL_ROWS)]

        # ---- persistent SBUF constants ----
        iota_sb = singles.tile([P, P], bf16)
        nc.sync.dma_start(out=iota_sb[:], in_=iota_d[:])
        ident_sb = singles.tile([P, P], f32)
        nc.sync.dma_start(out=ident_sb[:], in_=ident_d[:])
        eps_sb = singles.tile([P, 1], f32)
        nc.vector.memset(eps_sb[:], LN_EPS)

        def load_w(dram_t, name):
            tiles = []
            for kk in range(2):
                w_sb = singles.tile([P, D], f32, name=f"{name}_{kk}")
                nc.sync.dma_start(out=w_sb[:], in_=dram_t[kk * P:(kk + 1) * P, :])
                tiles.append(w_sb)
            return tiles

        w1a_rows = []
        for f in range(NF):
            w1a_r = singles.tile([EMB, D], f32, name=f"w1a_r{f}")
            nc.sync.dma_start(out=w1a_r[:], in_=w1a_d[f * EMB:(f + 1) * EMB, :])
            w1a_rows.append(w1a_r)

        def round_w(tiles, name):
            out = []
            for kk, w_sb in enumerate(tiles):
                w_r = singles.tile([P, D], bf16, name=f"{name}r_{kk}")
                nc.vector.tensor_copy(out=w_r[:], in_=w_sb[:])
                out.append(w_r)
            return out

        w1b_sb = round_w(load_w(w1b_d, "w1b"), "w1b")
        w2a_sb = round_w(load_w(w2a_d, "w2a"), "w2a")
        w2b_sb = round_w(load_w(w2b_d, "w2b"), "w2b")

        bias_sb = None
        if bias_d is not None:
            bias_tile = singles.tile([P, 8, D], f32)
            for r in range(8):
                nc.sync.dma_start(
                    out=bias_tile[:, r, :],
                    in_=bias_d[r].unsqueeze(0).to_broadcast([P, D]))
            bias_sb = [bias_tile[:, r, :] for r in range(8)]

        # NaN guard: skipped (-1) gather slots keep stale SBUF bytes that
        # feed the aggregation matmul multiplied by S==0; 0 * NaN would
        # poison PSUM, so zero the two G ring buffers once.
        for _ in range(2):
            Gz = gath_p.tile([P, Cg_max, D], bf16, tag="G")
            nc.vector.memset(Gz[:], 0.0)

        # =================================================================
        # Phase B: t tables  t[f] = emb_f @ w1a[64f:64f+64, :]   -> t_dram
        # =================================================================
        MT = 125  # 1000 = 8 * 125
        embT_p = ctx.enter_context(tc.tile_pool(name="embT_p", bufs=1))
        for f in range(NF):
            embT_sb = embT_p.tile([EMB, V], f32, tag="embT")
            nc.sync.dma_start(out=embT_sb[:], in_=embT_d[f])
            w_rows = w1a_rows[f][:]
            for j in range(V // MT):
                t_ps = ps_mm.tile([MT, D], f32, tag="tps", bufs=1)
                nc.tensor.matmul(out=t_ps[:],
                                 lhsT=embT_sb[:, j * MT:(j + 1) * MT],
                                 rhs=w_rows, start=True, stop=True)
                t_sb = work_p.tile([MT, D], bf16, tag="tsb")
                nc.vector.tensor_copy(out=t_sb[:], in_=t_ps[:])
                nc.sync.dma_start(
                    out=t_dram[f * V + j * MT:f * V + (j + 1) * MT, :],
                    in_=t_sb[:])

        # =================================================================
        # Phase C: z shard  z[n] = sum_f t[cmeta[n, f]]          -> z_shard
        # (sliced AllGather overlaps the build)
        # =================================================================
        sl_of_tile = {}
        for s, (rbase, nr) in enumerate(SL_ROWS):
            sl_of_tile[(rbase + nr - 1) // P] = s

        for gg in range(NTC):
            t0 = gg * CGT
            ntl = min(CGT, NT - t0)
            L = ntl * NF * P
            cm = meta_p.tile([128, CGT * NF * P // 16], i16, tag="cm")
            o16 = t0 * NF * P // 16
            nc.sync.dma_start(out=cm[:, :L // 16],
                              in_=cmeta_d[:, o16:o16 + L // 16])
            g4 = g4_p.tile([P, CGT * NF, D], bf16, tag="g4")
            for s in range(0, ntl * NF, 8):
                e = min(s + 8, ntl * NF)
                nc.gpsimd.dma_gather(
                    out_ap=g4[:, s:e, :], in_ap=t_dram[:, :],
                    idxs_ap=cm[:, s * 8:e * 8],
                    num_idxs=(e - s) * P, num_idxs_reg=(e - s) * P,
                    elem_size=D)
            for tt in range(ntl):
                t = t0 + tt
                valid = LAST_VALID if t == NT - 1 else P
                t01 = work_p.tile([P, 2, D], f32, tag="t01")
                nc.vector.tensor_add(out=t01[:],
                                     in0=g4[:, tt * NF:tt * NF + 2, :],
                                     in1=g4[:, tt * NF + 2:tt * NF + 4, :])
                z_t = work_p.tile([P, D], bf16, tag="z_t")
                nc.vector.tensor_add(out=z_t[:], in0=t01[:, 0, :],
                                     in1=t01[:, 1, :])
                nc.sync.dma_start(out=z_shard[t * P:t * P + valid, :],
                                  in_=z_t[:valid, :])
                if t in sl_of_tile:
                    s = sl_of_tile[t]
                    rbase, nr = SL_ROWS[s]
                    nc.gpsimd.collective_compute(
                        "AllGather", mybir.AluOpType.bypass,
                        replica_groups=groups,
                        ins=[z_shard[rbase:rbase + nr, :]],
                        outs=[z_full[s][:]])

        # =================================================================
        # Phases D/E: message passing + MLP layers
        # =================================================================
        def mp_layer(layer):
            tab_full = z_full if layer == 1 else z2_full
            tab_own = z_shard if layer == 1 else z2_shard
            wb_sb = w1b_sb if layer == 1 else w2b_sb
            ba_row, bb_row = (0, 1) if layer == 1 else (2, 3)
            g_row, b_row = (4, 5) if layer == 1 else (6, 7)

            for g in range(n_groups):
                calls = call_cols[g]
                g_lo = calls[0][1]
                g_hi = calls[-1][2]
                Cg = g_hi - g_lo
                if Cg == 0:
                    continue
                # --- metadata ---
                idx_sb = meta_p.tile([128, Cg_max * 8], i16, tag="idx")
                nc.sync.dma_start(
                    out=idx_sb[:, :Cg * 8],
                    in_=idx_d[:, g_lo * 8:g_hi * 8])
                drel = meta_p.tile([P, Cg_max], bf16, tag="drel")
                nc.sync.dma_start(out=drel[:, :Cg],
                                  in_=drel_d[:, g_lo:g_hi])
                # --- gather: <=8-chunk calls, one bucket each ---
                G = gath_p.tile([P, Cg_max, D], bf16, tag="G")
                for b, c_lo, c_hi, nv in calls:
                    nb = c_hi - c_lo
                    if nb == 0 or nv == 0:
                        continue
                    nc.gpsimd.dma_gather(
                        out_ap=G[:, c_lo - g_lo:c_hi - g_lo, :],
                        in_ap=tab_full[b][:, :],
                        idxs_ap=idx_sb[:, (c_lo - g_lo) * 8:(c_hi - g_lo) * 8],
                        num_idxs=nb * P, num_idxs_reg=nv, elem_size=D)
                # --- selection matrix for the whole group ---
                S = sel_p.tile([P, Cg_max, P], bf16, tag="S")
                nc.vector.tensor_tensor(
                    out=S[:, :Cg, :],
                    in0=drel[:, :Cg].unsqueeze(2).to_broadcast([P, Cg, P]),
                    in1=iota_sb[:].unsqueeze(1).to_broadcast([P, Cg, P]),
                    op=mybir.AluOpType.is_equal)

                for t in range(g * GSZ, min((g + 1) * GSZ, NT)):
                    valid = LAST_VALID if t == NT - 1 else P
                    ranges = tile_ranges[t]
                    ncols = sum(hi - lo for lo, hi in ranges)
                    # --- aggregate: agg[d, :] += S[:, c, d].T @ G[:, c, :] ---
                    agg_ps = ps_agg.tile([P, D], f32, tag="agg")
                    ci = 0
                    for lo, hi in ranges:
                        for c in range(lo - g_lo, hi - g_lo):
                            nc.tensor.matmul(out=agg_ps[:],
                                             lhsT=S[:, c, :], rhs=G[:, c, :],
                                             start=(ci == 0),
                                             stop=(ci == ncols - 1))
                            ci += 1
                    # --- u = relu(z_own + agg (+ba)) ---
                    zown = work_p.tile([P, D], bf16, tag="zown")
                    if valid < P:
                        nc.vector.memset(zown[:], 0.0)
                    nc.sync.dma_start(out=zown[:valid, :],
                                      in_=tab_own[t * P:t * P + valid, :])
                    u = work_p.tile([P, D], f32, tag="u")
                    nc.vector.tensor_add(out=u[:], in0=agg_ps[:], in1=zown[:])
                    if use_biases:
                        nc.vector.tensor_add(out=u[:], in0=u[:],
                                             in1=bias_sb[ba_row])
                    ur = work_p.tile([P, D], f32, tag="ur")
                    nc.scalar.activation(out=ur[:], in_=u[:], func=AF.Relu)
                    # --- v = u @ wb (+bb) ---
                    uT_ps = ps_tr.tile([P, 2, P], f32, tag="uT_ps")
                    for kk in range(2):
                        nc.tensor.transpose(out=uT_ps[:, kk, :],
                                            in_=ur[:, kk * P:(kk + 1) * P],
                                            identity=ident_sb[:])
                    uT = work_p.tile([P, 2, P], bf16, tag="uT")
                    nc.scalar.activation(out=uT[:], in_=uT_ps[:], func=AF.Copy)
                    v_ps = ps_mm.tile([P, D], f32, tag="v_ps")
                    for kk in range(2):
                        nc.tensor.matmul(out=v_ps[:],
                                         lhsT=uT[:, kk, :], rhs=wb_sb[kk][:],
                                         start=(kk == 0), stop=(kk == 1))
                    r = work_p.tile([P, D], f32, tag="r")
                    if use_biases:
                        nc.vector.tensor_add(out=r[:], in0=v_ps[:],
                                             in1=bias_sb[bb_row])
                        if layer == 1:
                            nc.vector.tensor_scalar_max(out=r[:], in0=r[:],
                                                        scalar1=0.0)
                    else:
                        nc.scalar.activation(
                            out=r[:], in_=v_ps[:],
                            func=AF.Relu if layer == 1 else AF.Copy)
                    # --- LayerNorm ---
                    stats = stat_p.tile([P, 6], f32, tag="stats")
                    nc.vector.bn_stats(out=stats[:], in_=r[:])
                    mv = stat_p.tile([P, 2], f32, tag="mv")
                    nc.vector.bn_aggr(out=mv[:], in_=stats[:])
                    nc.scalar.activation(out=mv[:, 1:2], in_=mv[:, 1:2],
                                         func=AF.Sqrt,
                                         bias=eps_sb[:], scale=1.0)
                    nc.vector.reciprocal(out=mv[:, 1:2], in_=mv[:, 1:2])
                    nm = stat_p.tile([P, 1], f32, tag="nm")
                    nc.vector.tensor_scalar(out=nm[:], in0=mv[:, 0:1],
                                            scalar1=mv[:, 1:2], scalar2=-1.0,
                                            op0=mybir.AluOpType.mult,
                                            op1=mybir.AluOpType.mult)
                    h = work_p.tile([P, D], f32, tag="h")
                    nc.scalar.activation(out=h[:], in_=r[:], func=AF.Identity,
                                         bias=nm[:], scale=mv[:, 1:2])
                    if use_ln_gb:
                        nc.vector.tensor_mul(out=h[:], in0=h[:],
                                             in1=bias_sb[g_row])
                        nc.vector.tensor_add(out=h[:], in0=h[:],
                                             in1=bias_sb[b_row])

                    if layer == 1:
                        # --- z2 = h @ w2a -> z2_shard ---
                        hT_ps = ps_tr.tile([P, 2, P], f32, tag="uT_ps")
                        for kk in range(2):
                            nc.tensor.transpose(out=hT_ps[:, kk, :],
                                                in_=h[:, kk * P:(kk + 1) * P],
                                                identity=ident_sb[:])
                        hT = work_p.tile([P, 2, P], bf16, tag="uT")
                        nc.scalar.activation(out=hT[:], in_=hT_ps[:],
                                             func=AF.Copy)
                        z2_ps = ps_mm.tile([P, D], f32, tag="v_ps")
                        for kk in range(2):
                            nc.tensor.matmul(out=z2_ps[:],
                                             lhsT=hT[:, kk, :],
                                             rhs=w2a_sb[kk][:],
                                             start=(kk == 0), stop=(kk == 1))
                        z2_sb = work_p.tile([P, D], bf16, tag="z2_sb")
                        nc.scalar.activation(out=z2_sb[:], in_=z2_ps[:],
                                             func=AF.Copy)
                        nc.sync.dma_start(
                            out=z2_shard[t * P:t * P + valid, :],
                            in_=z2_sb[:valid, :])
                        if t in sl_of_tile:
                            s = sl_of_tile[t]
                            rbase, nr = SL_ROWS[s]
                            nc.gpsimd.collective_compute(
                                "AllGather", mybir.AluOpType.bypass,
                                replica_groups=groups,
                                ins=[z2_shard[rbase:rbase + nr, :]],
                                outs=[z2_full[s][:]])
                    else:
                        nc.sync.dma_start(out=out_d[t * P:t * P + valid, :],
                                          in_=h[:valid, :])

        mp_layer(1)
        mp_layer(2)

    nc.compile()
    return nc


def get_program(Cb, call_cols, tile_ranges, **kw):
    key = (tuple(tuple(c) for c in Cb),
           tuple(tuple(c) for cs in call_cols for c in cs),
           tuple(sorted(kw.items())))
    if key not in _PROGRAM_CACHE:
        _PROGRAM_CACHE[key] = _build_program(Cb, call_cols, tile_ranges, **kw)
    return _PROGRAM_CACHE[key]


# ---------------------------------------------------------------------------
# Entry point
# ---------------------------------------------------------------------------
def kernel_with_results(x_cat, edge_index, emb0, emb1, emb2, emb3,
                        w1a, b1a, w1b, b1b, w2a, b2a, w2b, b2b,
                        ln1_g, ln1_b, ln2_g, ln2_b, trace=False):
    import ml_dtypes
    from concourse import bass_utils

    Cb, call_cols, tile_ranges, cmeta16, idx_meta, drel_meta = _prep_meta(
        x_cat, edge_index)

    f32 = np.float32
    embT = np.stack([np.ascontiguousarray(np.asarray(e, f32).T)
                     for e in (emb0, emb1, emb2, emb3)])
    w1a = np.ascontiguousarray(np.asarray(w1a, f32))
    w1b = np.ascontiguousarray(np.asarray(w1b, f32))
    w2a = np.ascontiguousarray(np.asarray(w2a, f32))
    w2b = np.ascontiguousarray(np.asarray(w2b, f32))

    biases = [np.asarray(b, f32) for b in (b1a, b1b, b2a, b2b)]
    lngb = [np.asarray(b, f32) for b in (ln1_g, ln1_b, ln2_g, ln2_b)]
    use_biases = any(np.any(b != 0.0) for b in biases)
    use_ln_gb = (np.any(lngb[0] != 1.0) or np.any(lngb[1] != 0.0)
                 or np.any(lngb[2] != 1.0) or np.any(lngb[3] != 0.0))

    iota_row = np.broadcast_to(
        np.arange(P).astype(ml_dtypes.bfloat16), (P, P)).copy()
    identity = np.eye(P, dtype=f32)

    nc = get_program(Cb, call_cols, tile_ranges, use_biases=use_biases,
                     use_ln_gb=use_ln_gb)

    in_maps = []
    for k in range(NC):
        m = {
            "embT": embT,
            "w1a": w1a, "w1b": w1b, "w2a": w2a, "w2b": w2b,
            "cmeta16": cmeta16[k],
            "idx_meta": idx_meta[k],
            "drel_meta": drel_meta[k],
            "iota_row": iota_row,
            "identity": identity,
        }
        if use_biases or use_ln_gb:
            m["biasrows"] = np.stack(biases + lngb)
        in_maps.append(m)

    res = bass_utils.run_bass_kernel_spmd(nc, in_maps, core_ids=list(range(NC)),
                                          trace=trace)
    out = np.concatenate([r["out"] for r in res.results], axis=0)
    return out.astype(np.float32), res


def kernel(**inputs):
    out, _ = kernel_with_results(**inputs)
    return out


# revision 14
# speedup vs baseline: 1.8938x; 1.8938x over previous
"""Trainium2 Bass kernel for a 2-layer categorical GIN encoder.

Graph: N=100000 nodes, E=1600000 edges, 256-dim features.

    x   = concat_i emb_i[x_cat[:, i]]                  # [N, 256]
    h1  = LN1(relu(relu((x + A x) @ w1a + b1a) @ w1b + b1b))
    out = LN2(relu((h1 + A h1) @ w2a + b2a) @ w2b + b2b)

where (A x)[d] = sum over edges (s -> d) of x[s].

Strategy (8 NeuronCores, SPMD):
  * Linearity trick: (x + Ax) @ w1a == z + A z with z = x @ w1a, and
    z[n] = sum_i t_i[x_cat[n, i]] where t_i = emb_i @ w1a[64i:64i+64, :].
    t tables are built on device (bf16); each core builds its z shard by
    dma_gather of t rows; shards are AllGathered in 4 row slices
    (overlapped with the z build) into a slice-major full table
    (global row = (slice, core, row)) so collective in/outs stay
    contiguous.
  * Edges are sorted by destination and sharded by destination range
    (12500 nodes per core) -> the aggregation needs no collective.
  * Per-edge gathers of z[src] use the InstDMAGatherAnt ucode whose
    ~1us SWDGE overhead is per *call*: calls cover up to 8 chunks
    (1024 edges, the SWDGE descriptor-ring cap) of a 4-tile group,
    one bucket (25000 permuted rows, int16 index limit) at a time,
    rotating over 4 SWDGE queues.  Edges are src-sorted inside each
    (tile, bucket) segment so descriptors read ascending addresses.
    Slots beyond a segment's max-over-cores edge count carry idx -1
    (skipped by the ucode, zero HBM traffic); slots between this
    core's count and the max carry idx 0 (row 0 fetch) so the valid
    count baked into the SPMD program is core-uniform.
  * Aggregation per tile: one-hot S[e, d] = (dst_rel[e] == d) built on
    DVE (bf16), agg += S.T @ G in PSUM (bf16 in, fp32 accumulate);
    padding slots have dst_rel -1 and select zero.
  * MLP per tile: PE transposes + bf16 weight matmuls; relu / LayerNorm
    application / PSUM->SBUF casts run on the Activation engine
    (func(scale*x+bias) with per-partition scale/bias) to unload DVE;
    bn_stats/bn_aggr remain on DVE.  Layer 1 also applies w2a so the
    second (also sliced) AllGather ships z2 = h1 @ w2a.
"""

import numpy as np

# ---------------------------------------------------------------------------
# Problem constants (hardcoded per contest contract)
# ---------------------------------------------------------------------------
N = 100000        # nodes
E = 1600000       # edges
D = 256           # feature dim (in = hidden = out)
EMB = 64          # per-field embedding dim
V = 1000          # categories per field
NF = 4            # categorical fields
NC = 8            # NeuronCores
P = 128           # partitions
LN_EPS = 1e-5

NSH = N // NC             # nodes per core (12500)
NT = (NSH + P - 1) // P   # node tiles per core (98)
LAST_VALID = NSH - (NT - 1) * P  # valid rows in last tile (84)

NBUK = 4                  # source buckets == allgather slices
GSZ = 4                   # dst tiles per gather group
CGT = 4                   # tiles per phase-C gather call
MAXC = 8                  # chunks per dma_gather call (1024-desc ring cap)
NSL = 4                   # allgather slices

# slice layout: tiles per slice -> local row ranges
_SL_TILES = [25, 25, 25, 23]
SL_ROWS = []              # (row_base, nrows) per slice
_rb = 0
for _s in range(NSL):
    _nr = min(_SL_TILES[_s] * P, NSH - _rb)
    SL_ROWS.append((_rb, _nr))
    _rb += _nr
assert _rb == NSH
GLOB_OFF = [0]
for _rb, _nr in SL_ROWS:
    GLOB_OFF.append(GLOB_OFF[-1] + _nr * NC)


def _permute_rows(node):
    """node id -> slice-major global row (slice, core, local row)."""
    k = node // NSH
    r = node - k * NSH
    out = np.zeros_like(node)
    for s, (rb, nr) in enumerate(SL_ROWS):
        m = (r >= rb) & (r < rb + nr)
        out = np.where(m, GLOB_OFF[s] + k * nr + (r - rb), out)
    return out


def _wrap_idx(flat):
    """[L] int16 (L % 16 == 0) -> [128, L//16] wrapped + replicated."""
    w = flat.reshape(-1, 16).T.copy()           # [16, L//16]
    return np.tile(w, (8, 1))                   # [128, L//16]


# ---------------------------------------------------------------------------
# Host-side preprocessing: shard + sort edges, build per-core metadata
# ---------------------------------------------------------------------------
def _prep_meta(x_cat, edge_index):
    import ml_dtypes

    src = np.asarray(edge_index[0], dtype=np.int64)
    dst = np.asarray(edge_index[1], dtype=np.int64)
    prow = _permute_rows(src)

    order = np.argsort(dst, kind="stable")
    dst_s = dst[order]
    prow_s = prow[order]
    bounds = np.searchsorted(dst_s, np.arange(NC + 1) * NSH)

    per_core = []
    counts_tb = np.zeros((NC, NT * NBUK), dtype=np.int64)
    for k in range(NC):
        lo, hi = bounds[k], bounds[k + 1]
        d_k = dst_s[lo:hi] - k * NSH
        p_k = prow_s[lo:hi]
        t_k = d_k // P
        rel = (d_k - t_k * P).astype(np.int64)
        b_k = np.searchsorted(GLOB_OFF, p_k, side="right") - 1
        key = t_k * NBUK + b_k
        # sort by (tile, bucket, src row) -> ascending gather addresses
        o2 = np.lexsort((p_k, key))
        key = key[o2]
        counts = np.bincount(key, minlength=NT * NBUK)
        counts_tb[k] = counts
        per_core.append((p_k[o2], rel[o2], key, counts))

    m_tb = counts_tb.max(axis=0)                       # [NT*NBUK] valid slots
    Cb = (-(-m_tb // P)).reshape(NT, NBUK)             # [NT, NBUK] chunks

    # global chunk column layout: group-major, bucket-major inside group
    n_groups = -(-NT // GSZ)
    col_start = np.zeros((NT, NBUK), dtype=np.int64)
    call_cols = []           # per group: [(b, col_lo, col_hi, n_valid)]
    cc = 0
    for g in range(n_groups):
        tiles = list(range(g * GSZ, min((g + 1) * GSZ, NT)))
        calls = []
        for b in range(NBUK):
            lo = cc
            for t in tiles:
                col_start[t, b] = cc
                cc += Cb[t, b]
            # split into <=MAXC-chunk calls (SWDGE ring cap).  nv =
            # num_idxs_reg = slots up to the last valid one: mid-call
            # padding gathers row 0 (mid-list -1 crashes the ucode);
            # only the call's tail may carry -1 (skipped).
            for s in range(lo, cc, MAXC):
                e = min(s + MAXC, cc)
                nv = 0
                for t in tiles:
                    a0, a1 = col_start[t, b], col_start[t, b] + Cb[t, b]
                    ov_lo, ov_hi = max(s, a0), min(e, a1)
                    if ov_hi <= ov_lo:
                        continue
                    mvv = int(m_tb[t * NBUK + b])
                    sl_lo = (ov_lo - a0) * P
                    sl_hi = (ov_hi - a0) * P
                    seg_valid = max(0, min(mvv, sl_hi) - sl_lo)
                    if seg_valid > 0:
                        nv = (ov_lo - s) * P + seg_valid
                if e > s:
                    calls.append((b, int(s), int(e), int(nv)))
        call_cols.append(calls)
    CT = int(cc)

    tile_ranges = [[(int(col_start[t, b]), int(col_start[t, b] + Cb[t, b]))
                    for b in range(NBUK) if Cb[t, b] > 0] for t in range(NT)]

    idx_meta, drel_meta = [], []
    for k in range(NC):
        p_k, rel_k, key_k, counts = per_core[k]
        starts = np.zeros(NT * NBUK, dtype=np.int64)
        np.cumsum(counts[:-1], out=starts[1:])
        j_seg = np.arange(len(p_k)) - starts[key_k]     # rank within (t,b)
        col = col_start.reshape(-1)[key_k] + j_seg // P
        slot = col * P + (j_seg % P)                    # global flat slot

        idxflat = np.full(CT * P, -1, dtype=np.int16)
        drelflat = np.full(CT * P, -1.0, dtype=np.float32)
        # padding below each call's valid cut fetches row 0 (idx 0);
        # everything after the cut stays -1 (skipped tail)
        for calls in call_cols:
            for b, c_lo, c_hi, nv in calls:
                base = c_lo * P
                idxflat[base:base + nv] = 0
        idxflat[slot] = (p_k - np.asarray(GLOB_OFF)[key_k % NBUK]).astype(np.int16)
        drelflat[slot] = rel_k

        idx_meta.append(_wrap_idx(idxflat))
        drel_meta.append(
            drelflat.reshape(CT, P).T.astype(ml_dtypes.bfloat16).copy())

    # phase-C index stream: slot (tile_in_call*NF + f)*128 + p
    x_cat = np.asarray(x_cat, dtype=np.int64)
    cmeta16 = []
    for k in range(NC):
        xc = x_cat[k * NSH:(k + 1) * NSH]               # [NSH, NF]
        rows = (xc + np.arange(NF)[None, :] * V).astype(np.int16)
        cm = np.zeros((NT, NF, P), dtype=np.int16)
        for t in range(NT):
            v = min(P, NSH - t * P)
            cm[t, :, :v] = rows[t * P:t * P + v].T
        cmeta16.append(_wrap_idx(cm.reshape(-1)))

    Cb_list = [[int(Cb[t, b]) for b in range(NBUK)] for t in range(NT)]
    return Cb_list, call_cols, tile_ranges, cmeta16, idx_meta, drel_meta


# ---------------------------------------------------------------------------
# Device program
# ---------------------------------------------------------------------------
_PROGRAM_CACHE = {}


def _build_program(Cb, call_cols, tile_ranges,
                   use_biases=False, use_ln_gb=False, num_cores=NC,
                   queue_map=None, compile_now=True):
    """Build (+ optionally compile) the SPMD Bass program.

    queue_map: emission-index -> SWDGE queue for the gather calls.  The
    scheduler may reorder gathers (e.g. hoist layer-2 bucket-b gathers
    that only need z2 slice b), and each DMASW semaphore (assigned
    final-order mod 8) must stay on one queue, so queues are assigned
    from the *final* order learned in a first pass.
    """
    import concourse.bacc as bacc
    import concourse.bass as bass
    import concourse.tile as tile
    from concourse import mybir

    f32 = mybir.dt.float32
    i16 = mybir.dt.int16
    bf16 = mybir.dt.bfloat16
    AF = mybir.ActivationFunctionType

    nc = bacc.Bacc("TRN2", target_bir_lowering=False, debug=False,
                   num_devices=num_cores, num_swdge_queues=4)

    CT = call_cols[-1][-1][2]
    n_groups = len(call_cols)
    NTC = -(-NT // CGT)
    Cg_max = max(calls[-1][2] - calls[0][1] for calls in call_cols)

    # ---- external I/O ----
    embT_d = nc.dram_tensor("embT", [NF, EMB, V], f32, kind="ExternalInput")
    w1a_d = nc.dram_tensor("w1a", [D, D], f32, kind="ExternalInput")
    w1b_d = nc.dram_tensor("w1b", [D, D], f32, kind="ExternalInput")
    w2a_d = nc.dram_tensor("w2a", [D, D], f32, kind="ExternalInput")
    w2b_d = nc.dram_tensor("w2b", [D, D], f32, kind="ExternalInput")
    cmeta_d = nc.dram_tensor("cmeta16", [128, NT * NF * P // 16], i16,
                             kind="ExternalInput")
    idx_d = nc.dram_tensor("idx_meta", [128, CT * P // 16], i16,
                           kind="ExternalInput")
    drel_d = nc.dram_tensor("drel_meta", [P, CT], bf16, kind="ExternalInput")
    iota_d = nc.dram_tensor("iota_row", [P, P], bf16, kind="ExternalInput")
    ident_d = nc.dram_tensor("identity", [P, P], f32, kind="ExternalInput")
    bias_d = None
    if use_biases or use_ln_gb:
        # rows: b1a, b1b, b2a, b2b, ln1_g, ln1_b, ln2_g, ln2_b
        bias_d = nc.dram_tensor("biasrows", [8, D], f32, kind="ExternalInput")
    out_d = nc.dram_tensor("out", [NSH, D], f32, kind="ExternalOutput")

    groups = [list(range(num_cores))]

    from contextlib import ExitStack

    qctr = [0]
    emit_names = []

    def next_q():
        i = qctr[0]
        qctr[0] += 1
        return queue_map[i] if queue_map is not None else 0

    with tile.TileContext(nc) as tc, ExitStack() as ctx:
        singles = ctx.enter_context(tc.tile_pool(name="singles", bufs=1))
        dram = ctx.enter_context(tc.tile_pool(name="dram", bufs=1, space="DRAM"))
        meta_p = ctx.enter_context(tc.tile_pool(name="meta", bufs=4))
        gath_p = ctx.enter_context(tc.tile_pool(name="gath", bufs=2))
        g4_p = ctx.enter_context(tc.tile_pool(name="g4", bufs=2))
        sel_p = ctx.enter_context(tc.tile_pool(name="sel", bufs=2))
        work_p = ctx.enter_context(tc.tile_pool(name="work", bufs=4))
        stat_p = ctx.enter_context(tc.tile_pool(name="stat", bufs=4))
        ps_agg = ctx.enter_context(tc.tile_pool(name="ps_agg", bufs=3, space="PSUM"))
        ps_tr = ctx.enter_context(tc.tile_pool(name="ps_tr", bufs=2, space="PSUM"))
        ps_mm = ctx.enter_context(tc.tile_pool(name="ps_mm", bufs=2, space="PSUM"))

        # ---- internal DRAM tables ----
        t_dram = dram.tile([NF * V, D], bf16)
        z_shard = dram.tile([NSH, D], bf16)
        z2_shard = dram.tile([NSH, D], bf16)
        z_full = [dram.tile([nr * num_cores, D], bf16, addr_space="Shared",
                            name=f"z_full_{s}")
                  for s, (rb, nr) in enumerate(SL_ROWS)]
        z2_full = [dram.tile([nr * num_cores, D], bf16, addr_space="Shared",
                             name=f"z2_full_{s}")
                   for s, (rb, nr) in enumerate(SL_ROWS)]

        # ---- persistent SBUF constants ----
        iota_sb = singles.tile([P, P], bf16)
        nc.sync.dma_start(out=iota_sb[:], in_=iota_d[:])
        ident_sb = singles.tile([P, P], f32)
        nc.sync.dma_start(out=ident_sb[:], in_=ident_d[:])
        eps_sb = singles.tile([P, 1], f32)
        nc.vector.memset(eps_sb[:], LN_EPS)

        def load_w(dram_t, name):
            tiles = []
            for kk in range(2):
                w_sb = singles.tile([P, D], f32, name=f"{name}_{kk}")
                nc.sync.dma_start(out=w_sb[:], in_=dram_t[kk * P:(kk + 1) * P, :])
                tiles.append(w_sb)
            return tiles

        w1a_rows = []
        for f in range(NF):
            w1a_r = singles.tile([EMB, D], f32, name=f"w1a_r{f}")
            nc.sync.dma_start(out=w1a_r[:], in_=w1a_d[f * EMB:(f + 1) * EMB, :])
            w1a_rows.append(w1a_r)

        def round_w(tiles, name):
            out = []
            for kk, w_sb in enumerate(tiles):
                w_r = singles.tile([P, D], bf16, name=f"{name}r_{kk}")
                nc.vector.tensor_copy(out=w_r[:], in_=w_sb[:])
                out.append(w_r)
            return out

        w1b_sb = round_w(load_w(w1b_d, "w1b"), "w1b")
        w2a_sb = round_w(load_w(w2a_d, "w2a"), "w2a")
        w2b_sb = round_w(load_w(w2b_d, "w2b"), "w2b")

        bias_sb = None
        if bias_d is not None:
            bias_tile = singles.tile([P, 8, D], f32)
            for r in range(8):
                nc.sync.dma_start(
                    out=bias_tile[:, r, :],
                    in_=bias_d[r].unsqueeze(0).to_broadcast([P, D]))
            bias_sb = [bias_tile[:, r, :] for r in range(8)]

        # NaN guard: skipped (-1) gather slots keep stale SBUF bytes that
        # feed the aggregation matmul multiplied by S==0; 0 * NaN would
        # poison PSUM, so zero the two G ring buffers once.
        for _ in range(2):
            Gz = gath_p.tile([P, Cg_max, D], bf16, tag="G")
            nc.vector.memset(Gz[:], 0.0)

        # =================================================================
        # Phase B: t tables  t[f] = emb_f @ w1a[64f:64f+64, :]   -> t_dram
        # =================================================================
        MT = 125  # 1000 = 8 * 125
        embT_p = ctx.enter_context(tc.tile_pool(name="embT_p", bufs=1))
        for f in range(NF):
            embT_sb = embT_p.tile([EMB, V], f32, tag="embT")
            nc.sync.dma_start(out=embT_sb[:], in_=embT_d[f])
            w_rows = w1a_rows[f][:]
            for j in range(V // MT):
                t_ps = ps_mm.tile([MT, D], f32, tag="tps", bufs=1)
                nc.tensor.matmul(out=t_ps[:],
                                 lhsT=embT_sb[:, j * MT:(j + 1) * MT],
                                 rhs=w_rows, start=True, stop=True)
                t_sb = work_p.tile([MT, D], bf16, tag="tsb")
                nc.vector.tensor_copy(out=t_sb[:], in_=t_ps[:])
                nc.sync.dma_start(
                    out=t_dram[f * V + j * MT:f * V + (j + 1) * MT, :],
                    in_=t_sb[:])

        # =================================================================
        # Phase C: z shard  z[n] = sum_f t[cmeta[n, f]]          -> z_shard
        # (sliced AllGather overlaps the build)
        # =================================================================
        sl_of_tile = {}
        for s, (rbase, nr) in enumerate(SL_ROWS):
            sl_of_tile[(rbase + nr - 1) // P] = s

        for gg in range(NTC):
            t0 = gg * CGT
            ntl = min(CGT, NT - t0)
            L = ntl * NF * P
            cm = meta_p.tile([128, CGT * NF * P // 16], i16, tag="cm")
            o16 = t0 * NF * P // 16
            nc.sync.dma_start(out=cm[:, :L // 16],
                              in_=cmeta_d[:, o16:o16 + L // 16])
            g4 = g4_p.tile([P, CGT * NF, D], bf16, tag="g4")
            for s in range(0, ntl * NF, 8):
                e = min(s + 8, ntl * NF)
                inst = nc.gpsimd.dma_gather(
                    out_ap=g4[:, s:e, :], in_ap=t_dram[:, :],
                    idxs_ap=cm[:, s * 8:e * 8],
                    num_idxs=(e - s) * P, num_idxs_reg=(e - s) * P,
                    elem_size=D, queue_num=next_q())
                emit_names.append(inst.ins.name)
            for tt in range(ntl):
                t = t0 + tt
                valid = LAST_VALID if t == NT - 1 else P
                t01 = work_p.tile([P, 2, D], f32, tag="t01")
                nc.vector.tensor_add(out=t01[:],
                                     in0=g4[:, tt * NF:tt * NF + 2, :],
                                     in1=g4[:, tt * NF + 2:tt * NF + 4, :])
                z_t = work_p.tile([P, D], bf16, tag="z_t")
                nc.vector.tensor_add(out=z_t[:], in0=t01[:, 0, :],
                                     in1=t01[:, 1, :])
                nc.sync.dma_start(out=z_shard[t * P:t * P + valid, :],
                                  in_=z_t[:valid, :])
                if t in sl_of_tile:
                    s = sl_of_tile[t]
                    rbase, nr = SL_ROWS[s]
                    nc.gpsimd.collective_compute(
                        "AllGather", mybir.AluOpType.bypass,
                        replica_groups=groups,
                        ins=[z_shard[rbase:rbase + nr, :]],
                        outs=[z_full[s][:]])

        # =================================================================
        # Phases D/E: message passing + MLP layers
        # =================================================================
        def mp_layer(layer):
            tab_full = z_full if layer == 1 else z2_full
            tab_own = z_shard if layer == 1 else z2_shard
            wb_sb = w1b_sb if layer == 1 else w2b_sb
            ba_row, bb_row = (0, 1) if layer == 1 else (2, 3)
            g_row, b_row = (4, 5) if layer == 1 else (6, 7)

            for g in range(n_groups):
                calls = call_cols[g]
                g_lo = calls[0][1]
                g_hi = calls[-1][2]
                Cg = g_hi - g_lo
                if Cg == 0:
                    continue
                # --- metadata ---
                idx_sb = meta_p.tile([128, Cg_max * 8], i16, tag="idx")
                nc.sync.dma_start(
                    out=idx_sb[:, :Cg * 8],
                    in_=idx_d[:, g_lo * 8:g_hi * 8])
                drel = meta_p.tile([P, Cg_max], bf16, tag="drel")
                nc.sync.dma_start(out=drel[:, :Cg],
                                  in_=drel_d[:, g_lo:g_hi])
                # --- gather: <=8-chunk calls, one bucket each ---
                G = gath_p.tile([P, Cg_max, D], bf16, tag="G")
                for b, c_lo, c_hi, nv in calls:
                    nb = c_hi - c_lo
                    if nb == 0 or nv == 0:
                        continue
                    inst = nc.gpsimd.dma_gather(
                        out_ap=G[:, c_lo - g_lo:c_hi - g_lo, :],
                        idxs_ap=idx_sb[:, (c_lo - g_lo) * 8:(c_hi - g_lo) * 8],
                        in_ap=tab_full[b][:, :],
                        num_idxs=nb * P, num_idxs_reg=nv, elem_size=D,
                        queue_num=next_q())
                    emit_names.append(inst.ins.name)
                # --- selection matrix for the whole group ---
                S = sel_p.tile([P, Cg_max, P], bf16, tag="S")
                nc.vector.tensor_tensor(
                    out=S[:, :Cg, :],
                    in0=drel[:, :Cg].unsqueeze(2).to_broadcast([P, Cg, P]),
                    in1=iota_sb[:].unsqueeze(1).to_broadcast([P, Cg, P]),
                    op=mybir.AluOpType.is_equal)

                for t in range(g * GSZ, min((g + 1) * GSZ, NT)):
                    valid = LAST_VALID if t == NT - 1 else P
                    ranges = tile_ranges[t]
                    ncols = sum(hi - lo for lo, hi in ranges)
                    # --- aggregate: agg[d, :] += S[:, c, d].T @ G[:, c, :] ---
                    agg_ps = ps_agg.tile([P, D], f32, tag="agg")
                    ci = 0
                    for lo, hi in ranges:
                        for c in range(lo - g_lo, hi - g_lo):
                            nc.tensor.matmul(out=agg_ps[:],
                                             lhsT=S[:, c, :], rhs=G[:, c, :],
                                             start=(ci == 0),
                                             stop=(ci == ncols - 1))
                            ci += 1
                    # --- u = relu(z_own + agg (+ba)) ---
                    zown = work_p.tile([P, D], bf16, tag="zown")
                    if valid < P:
                        nc.vector.memset(zown[:], 0.0)
                    nc.sync.dma_start(out=zown[:valid, :],
                                      in_=tab_own[t * P:t * P + valid, :])
                    u = work_p.tile([P, D], f32, tag="u")
                    nc.vector.tensor_add(out=u[:], in0=agg_ps[:], in1=zown[:])
                    if use_biases:
                        nc.vector.tensor_add(out=u[:], in0=u[:],
                                             in1=bias_sb[ba_row])
                    ur = work_p.tile([P, D], f32, tag="ur")
                    nc.scalar.activation(out=ur[:], in_=u[:], func=AF.Relu)
                    # --- v = u @ wb (+bb) ---
                    uT_ps = ps_tr.tile([P, 2, P], f32, tag="uT_ps")
                    for kk in range(2):
                        nc.tensor.transpose(out=uT_ps[:, kk, :],
                                            in_=ur[:, kk * P:(kk + 1) * P],
                                            identity=ident_sb[:])
                    uT = work_p.tile([P, 2, P], bf16, tag="uT")
                    nc.scalar.activation(out=uT[:], in_=uT_ps[:], func=AF.Copy)
                    v_ps = ps_mm.tile([P, D], f32, tag="v_ps")
                    for kk in range(2):
                        nc.tensor.matmul(out=v_ps[:],
                                         lhsT=uT[:, kk, :], rhs=wb_sb[kk][:],
                                         start=(kk == 0), stop=(kk == 1))
                    r = work_p.tile([P, D], f32, tag="r")
                    if use_biases:
                        nc.vector.tensor_add(out=r[:], in0=v_ps[:],
                                             in1=bias_sb[bb_row])
                        if layer == 1:
                            nc.vector.tensor_scalar_max(out=r[:], in0=r[:],
                                                        scalar1=0.0)
                    else:
                        nc.scalar.activation(
                            out=r[:], in_=v_ps[:],
                            func=AF.Relu if layer == 1 else AF.Copy)
                    # --- LayerNorm ---
                    stats = stat_p.tile([P, 6], f32, tag="stats")
                    nc.vector.bn_stats(out=stats[:], in_=r[:])
                    mv = stat_p.tile([P, 2], f32, tag="mv")
                    nc.vector.bn_aggr(out=mv[:], in_=stats[:])
                    nc.scalar.activation(out=mv[:, 1:2], in_=mv[:, 1:2],
                                         func=AF.Sqrt,
                                         bias=eps_sb[:], scale=1.0)
                    nc.vector.reciprocal(out=mv[:, 1:2], in_=mv[:, 1:2])
                    nm = stat_p.tile([P, 1], f32, tag="nm")
                    nc.vector.tensor_scalar(out=nm[:], in0=mv[:, 0:1],
                                            scalar1=mv[:, 1:2], scalar2=-1.0,
                                            op0=mybir.AluOpType.mult,
                                            op1=mybir.AluOpType.mult)
                    h = work_p.tile([P, D], f32, tag="h")
                    nc.scalar.activation(out=h[:], in_=r[:], func=AF.Identity,
                                         bias=nm[:], scale=mv[:, 1:2])
                    if use_ln_gb:
                        nc.vector.tensor_mul(out=h[:], in0=h[:],
                                             in1=bias_sb[g_row])
                        nc.vector.tensor_add(out=h[:], in0=h[:],
                                             in1=bias_sb[b_row])

                    if layer == 1:
                        # --- z2 = h @ w2a -> z2_shard ---
                        hT_ps = ps_tr.tile([P, 2, P], f32, tag="uT_ps")
                        for kk in range(2):
                            nc.tensor.transpose(out=hT_ps[:, kk, :],
                                                in_=h[:, kk * P:(kk + 1) * P],
                                                identity=ident_sb[:])
                        hT = work_p.tile([P, 2, P], bf16, tag="uT")
                        nc.scalar.activation(out=hT[:], in_=hT_ps[:],
                                             func=AF.Copy)
                        z2_ps = ps_mm.tile([P, D], f32, tag="v_ps")
                        for kk in range(2):
                            nc.tensor.matmul(out=z2_ps[:],
                                             lhsT=hT[:, kk, :],
                                             rhs=w2a_sb[kk][:],
                                             start=(kk == 0), stop=(kk == 1))
                        z2_sb = work_p.tile([P, D], bf16, tag="z2_sb")
                        nc.scalar.activation(out=z2_sb[:], in_=z2_ps[:],
                                             func=AF.Copy)
                        nc.sync.dma_start(
                            out=z2_shard[t * P:t * P + valid, :],
                            in_=z2_sb[:valid, :])
                        if t in sl_of_tile:
                            s = sl_of_tile[t]
                            rbase, nr = SL_ROWS[s]
                            nc.gpsimd.collective_compute(
                                "AllGather", mybir.AluOpType.bypass,
                                replica_groups=groups,
                                ins=[z2_shard[rbase:rbase + nr, :]],
                                outs=[z2_full[s][:]])
                    else:
                        nc.sync.dma_start(out=out_d[t * P:t * P + valid, :],
                                          in_=h[:valid, :])

        mp_layer(1)
        mp_layer(2)

    from concourse.tile_sem_assignment import DMAInst
    final_names = []
    for fn in nc.m.functions:
        for blk in fn.blocks:
            for inst in blk.instructions:
                if (isinstance(inst, DMAInst)
                        and inst.engine == mybir.EngineType.Pool):
                    final_names.append(inst.name)
    emit_idx = {nm: i for i, nm in enumerate(emit_names)}
    qmap = [0] * len(emit_names)
    aligned = True
    for pos, nm in enumerate(final_names):
        qmap[emit_idx[nm]] = pos % 4
        if queue_map is not None and queue_map[emit_idx[nm]] != pos % 4:
            aligned = False
    if compile_now:
        nc.compile()
    return nc, qmap, aligned


def get_program(Cb, call_cols, tile_ranges, **kw):
    key = (tuple(tuple(c) for c in Cb),
           tuple(tuple(c) for cs in call_cols for c in cs),
           tuple(sorted(kw.items())))
    if key not in _PROGRAM_CACHE:
        _, qmap, _ = _build_program(Cb, call_cols, tile_ranges,
                                    compile_now=False, **kw)
        for _ in range(3):
            nc, qmap2, aligned = _build_program(
                Cb, call_cols, tile_ranges, queue_map=qmap,
                compile_now=True, **kw)
            if aligned:
                break
            qmap = qmap2
        assert aligned, "SWDGE queue assignment did not converge"
        _PROGRAM_CACHE[key] = nc
    return _PROGRAM_CACHE[key]


# ---------------------------------------------------------------------------
# Entry point
# ---------------------------------------------------------------------------
def kernel_with_results(x_cat, edge_index, emb0, emb1, emb2, emb3,
                        w1a, b1a, w1b, b1b, w2a, b2a, w2b, b2b,
                        ln1_g, ln1_b, ln2_g, ln2_b, trace=False):
    import ml_dtypes
    from concourse import bass_utils

    Cb, call_cols, tile_ranges, cmeta16, idx_meta, drel_meta = _prep_meta(
        x_cat, edge_index)

    f32 = np.float32
    embT = np.stack([np.ascontiguousarray(np.asarray(e, f32).T)
                     for e in (emb0, emb1, emb2, emb3)])
    w1a = np.ascontiguousarray(np.asarray(w1a, f32))
    w1b = np.ascontiguousarray(np.asarray(w1b, f32))
    w2a = np.ascontiguousarray(np.asarray(w2a, f32))
    w2b = np.ascontiguousarray(np.asarray(w2b, f32))

    biases = [np.asarray(b, f32) for b in (b1a, b1b, b2a, b2b)]
    lngb = [np.asarray(b, f32) for b in (ln1_g, ln1_b, ln2_g, ln2_b)]
    use_biases = any(np.any(b != 0.0) for b in biases)
    use_ln_gb = (np.any(lngb[0] != 1.0) or np.any(lngb[1] != 0.0)
                 or np.any(lngb[2] != 1.0) or np.any(lngb[3] != 0.0))

    iota_row = np.broadcast_to(
        np.arange(P).astype(ml_dtypes.bfloat16), (P, P)).copy()
    identity = np.eye(P, dtype=f32)

    nc = get_program(Cb, call_cols, tile_ranges, use_biases=use_biases,
                     use_ln_gb=use_ln_gb)

    in_maps = []
    for k in range(NC):
        m = {
            "embT": embT,
            "w1a": w1a, "w1b": w1b, "w2a": w2a, "w2b": w2b,
            "cmeta16": cmeta16[k],
            "idx_meta": idx_meta[k],
            "drel_meta": drel_meta[k],
            "iota_row": iota_row,
            "identity": identity,
        }
        if use_biases or use_ln_gb:
            m["biasrows"] = np.stack(biases + lngb)
        in_maps.append(m)

    res = bass_utils.run_bass_kernel_spmd(nc, in_maps, core_ids=list(range(NC)),
                                          trace=trace)
    out = np.concatenate([r["out"] for r in res.results], axis=0)
    return out.astype(np.float32), res


def kernel(**inputs):
    out, _ = kernel_with_results(**inputs)
    return out


# revision 17
# speedup vs baseline: 2.7289x; 1.4410x over previous
"""Trainium2 Bass kernel for a 2-layer categorical GIN encoder.

Graph: N=100000 nodes, E=1600000 edges, 256-dim features.

    x   = concat_i emb_i[x_cat[:, i]]                  # [N, 256]
    h1  = LN1(relu(relu((x + A x) @ w1a + b1a) @ w1b + b1b))
    out = LN2(relu((h1 + A h1) @ w2a + b2a) @ w2b + b2b)

where (A x)[d] = sum over edges (s -> d) of x[s].

Strategy (8 NeuronCores, SPMD):
  * Linearity trick: (x + Ax) @ w1a == z + A z with z = x @ w1a, and
    z[n] = sum_i t_i[x_cat[n, i]] where t_i = emb_i @ w1a[64i:64i+64, :].
    t tables are built on device (bf16); each core builds its z shard by
    dma_gather of t rows; shards are AllGathered in 4 row slices
    (overlapped with the z build) into a slice-major full table
    (global row = (slice, core, row)) so collective in/outs stay
    contiguous.
  * Edges are sorted by destination and sharded by destination range
    (12500 nodes per core) -> the aggregation needs no collective.
  * Per-edge gathers of z[src] use the InstDMAGatherAnt ucode whose
    ~1us SWDGE overhead is per *call*: calls cover up to 8 chunks
    (1024 edges, the SWDGE descriptor-ring cap) of a 4-tile group,
    one bucket (25000 permuted rows, int16 index limit) at a time,
    rotating over 4 SWDGE queues.  Edges are src-sorted inside each
    (tile, bucket) segment so descriptors read ascending addresses.
    Slots beyond a segment's max-over-cores edge count carry idx -1
    (skipped by the ucode, zero HBM traffic); slots between this
    core's count and the max carry idx 0 (row 0 fetch) so the valid
    count baked into the SPMD program is core-uniform.
  * Aggregation per tile: one-hot S[e, d] = (dst_rel[e] == d) built on
    DVE (bf16), agg += S.T @ G in PSUM (bf16 in, fp32 accumulate);
    padding slots have dst_rel -1 and select zero.
  * MLP per tile: PE transposes + bf16 weight matmuls; relu / LayerNorm
    application / PSUM->SBUF casts run on the Activation engine
    (func(scale*x+bias) with per-partition scale/bias) to unload DVE;
    bn_stats/bn_aggr remain on DVE.  Layer 1 also applies w2a so the
    second (also sliced) AllGather ships z2 = h1 @ w2a.
"""

import numpy as np

# ---------------------------------------------------------------------------
# Problem constants (hardcoded per contest contract)
# ---------------------------------------------------------------------------
N = 100000        # nodes
E = 1600000       # edges
D = 256           # feature dim (in = hidden = out)
EMB = 64          # per-field embedding dim
V = 1000          # categories per field
NF = 4            # categorical fields
NC = 8            # NeuronCores
P = 128           # partitions
LN_EPS = 1e-5

NSH = N // NC             # nodes per core (12500)
NT = (NSH + P - 1) // P   # node tiles per core (98)
LAST_VALID = NSH - (NT - 1) * P  # valid rows in last tile (84)

NBUK = 4                  # source buckets == allgather slices
GSZ = 4                   # dst tiles per gather group
CGT = 4                   # tiles per phase-C gather call
MAXC = 8                  # chunks per dma_gather call (1024-desc ring cap)
NSL = 4                   # allgather slices

# slice layout: tiles per slice -> local row ranges
_SL_TILES = [25, 25, 25, 23]
SL_ROWS = []              # (row_base, nrows) per slice
_rb = 0
for _s in range(NSL):
    _nr = min(_SL_TILES[_s] * P, NSH - _rb)
    SL_ROWS.append((_rb, _nr))
    _rb += _nr
assert _rb == NSH
GLOB_OFF = [0]
for _rb, _nr in SL_ROWS:
    GLOB_OFF.append(GLOB_OFF[-1] + _nr * NC)


def _permute_rows(node):
    """node id -> slice-major global row (slice, core, local row)."""
    k = node // NSH
    r = node - k * NSH
    out = np.zeros_like(node)
    for s, (rb, nr) in enumerate(SL_ROWS):
        m = (r >= rb) & (r < rb + nr)
        out = np.where(m, GLOB_OFF[s] + k * nr + (r - rb), out)
    return out


def _wrap_idx(flat):
    """[L] int16 (L % 16 == 0) -> [128, L//16] wrapped + replicated."""
    w = flat.reshape(-1, 16).T.copy()           # [16, L//16]
    return np.tile(w, (8, 1))                   # [128, L//16]


# ---------------------------------------------------------------------------
# Host-side preprocessing: shard + sort edges, build per-core metadata
# ---------------------------------------------------------------------------
def _prep_meta(x_cat, edge_index):
    import ml_dtypes

    src = np.asarray(edge_index[0], dtype=np.int64)
    dst = np.asarray(edge_index[1], dtype=np.int64)
    prow = _permute_rows(src)

    order = np.argsort(dst, kind="stable")
    dst_s = dst[order]
    prow_s = prow[order]
    bounds = np.searchsorted(dst_s, np.arange(NC + 1) * NSH)

    n_groups = -(-NT // GSZ)
    per_core = []
    counts_tb = np.zeros((NC, NT * NBUK), dtype=np.int64)
    for k in range(NC):
        lo, hi = bounds[k], bounds[k + 1]
        d_k = dst_s[lo:hi] - k * NSH
        p_k = prow_s[lo:hi]
        t_k = d_k // P
        rel = (d_k - t_k * P).astype(np.int64)
        b_k = np.searchsorted(GLOB_OFF, p_k, side="right") - 1
        g_k = t_k // GSZ
        # region key: (group, bucket); order inside: (tile, src row)
        key = t_k * NBUK + b_k
        rkey = (g_k * NBUK + b_k) * (GSZ + 1) + (t_k - g_k * GSZ)
        o2 = np.lexsort((p_k, rkey))
        key = key[o2]
        counts = np.bincount(key, minlength=NT * NBUK)
        counts_tb[k] = counts
        per_core.append((p_k[o2], rel[o2], (t_k - g_k * GSZ)[o2],
                         (g_k * NBUK + b_k)[o2], key))

    # per-(group,bucket) region sizes: max total edges over cores
    gb_count = np.zeros((NC, n_groups * NBUK), dtype=np.int64)
    for k in range(NC):
        c = counts_tb[k].reshape(NT, NBUK)
        for g in range(n_groups):
            gb_count[k, g * NBUK:(g + 1) * NBUK] = (
                c[g * GSZ:(g + 1) * GSZ].sum(axis=0))
    gb_max = gb_count.max(axis=0)                     # [n_groups*NBUK]
    gb_chunks = -(-gb_max // P)

    # per-(tile,bucket) start offsets inside the region, per core, plus
    # the union chunk range over cores for the tile matmuls
    tb_lo = np.full((NT, NBUK), 1 << 60, dtype=np.int64)
    tb_hi = np.zeros((NT, NBUK), dtype=np.int64)
    core_tb_start = np.zeros((NC, NT, NBUK), dtype=np.int64)
    for k in range(NC):
        c = counts_tb[k].reshape(NT, NBUK)
        for g in range(n_groups):
            tiles = list(range(g * GSZ, min((g + 1) * GSZ, NT)))
            for b in range(NBUK):
                off = 0
                for t in tiles:
                    core_tb_start[k, t, b] = off
                    lo_ch = off // P
                    off += c[t, b]
                    hi_ch = -(-off // P) if c[t, b] > 0 else lo_ch
                    tb_lo[t, b] = min(tb_lo[t, b], lo_ch)
                    tb_hi[t, b] = max(tb_hi[t, b], hi_ch)

    # global chunk columns: regions in (group, bucket) order
    reg_col = np.zeros(n_groups * NBUK, dtype=np.int64)
    cc = 0
    call_cols = []           # per group: [(b, col_lo, col_hi, n_valid)]
    for g in range(n_groups):
        calls = []
        for b in range(NBUK):
            gi = g * NBUK + b
            reg_col[gi] = cc
            nch = int(gb_chunks[gi])
            nv_total = int(gb_max[gi])
            for s in range(0, nch, MAXC):
                e = min(s + MAXC, nch)
                nv = max(0, min(nv_total - s * P, (e - s) * P))
                if nv > 0:
                    calls.append((b, int(cc + s), int(cc + e), int(nv)))
            cc += nch
        call_cols.append(calls)
    CT = int(cc)

    # per-tile chunk ranges (union over cores) + in-group tile index
    tile_ranges = []
    for t in range(NT):
        g = t // GSZ
        rr = []
        for b in range(NBUK):
            gi = g * NBUK + b
            if tb_hi[t, b] > tb_lo[t, b]:
                rr.append((int(reg_col[gi] + tb_lo[t, b]),
                           int(reg_col[gi] + tb_hi[t, b])))
        tile_ranges.append(rr)

    idx_meta, drel_meta = [], []
    for k in range(NC):
        p_k, rel_k, tin_k, gi_k, key_k = per_core[k]
        # rank within each (t,b) run of the region-sorted order
        run_first = np.zeros(NT * NBUK, dtype=np.int64)
        rs = np.concatenate([[0], np.flatnonzero(np.diff(key_k) != 0) + 1])
        run_first[key_k[rs]] = rs
        j_seg = np.arange(len(p_k)) - run_first[key_k]
        t_all = key_k // NBUK
        b_all = key_k % NBUK
        off_in_reg = core_tb_start[k, t_all, b_all] + j_seg
        slot = reg_col[gi_k] * P + off_in_reg           # compact flat slot

        idxflat = np.full(CT * P, -1, dtype=np.int16)
        drelflat = np.full(CT * P, -1.0, dtype=np.float32)
        # pad [n_core, n_max) of each region with idx 0 (uniform reg)
        for gi in range(n_groups * NBUK):
            n_k = int(gb_count[k, gi])
            n_mx = int(gb_max[gi])
            if n_mx > n_k:
                base = reg_col[gi] * P
                idxflat[base + n_k:base + n_mx] = 0
        idxflat[slot] = (p_k - np.asarray(GLOB_OFF)[b_all]).astype(np.int16)
        drelflat[slot] = rel_k + P * tin_k              # tile-in-group coded

        idx_meta.append(_wrap_idx(idxflat))
        drel_meta.append(
            drelflat.reshape(CT, P).T.astype(np.float16).copy())

    # phase-C index stream: slot (tile_in_call*NF + f)*128 + p
    x_cat = np.asarray(x_cat, dtype=np.int64)
    cmeta16 = []
    for k in range(NC):
        xc = x_cat[k * NSH:(k + 1) * NSH]               # [NSH, NF]
        rows = (xc + np.arange(NF)[None, :] * V).astype(np.int16)
        cm = np.zeros((NT, NF, P), dtype=np.int16)
        for t in range(NT):
            v = min(P, NSH - t * P)
            cm[t, :, :v] = rows[t * P:t * P + v].T
        cmeta16.append(_wrap_idx(cm.reshape(-1)))

    Cb_list = [[int(x) for x in row] for row in gb_count.max(axis=0).reshape(
        n_groups, NBUK)]
    return Cb_list, call_cols, tile_ranges, cmeta16, idx_meta, drel_meta


# ---------------------------------------------------------------------------
# Device program
# ---------------------------------------------------------------------------
_PROGRAM_CACHE = {}


def _build_program(Cb, call_cols, tile_ranges,
                   use_biases=False, use_ln_gb=False, num_cores=NC,
                   queue_map=None, compile_now=True):
    """Build (+ optionally compile) the SPMD Bass program.

    queue_map: emission-index -> SWDGE queue for the gather calls.  The
    scheduler may reorder gathers (e.g. hoist layer-2 bucket-b gathers
    that only need z2 slice b), and each DMASW semaphore (assigned
    final-order mod 8) must stay on one queue, so queues are assigned
    from the *final* order learned in a first pass.
    """
    import concourse.bacc as bacc
    import concourse.bass as bass
    import concourse.tile as tile
    from concourse import mybir

    f32 = mybir.dt.float32
    i16 = mybir.dt.int16
    bf16 = mybir.dt.bfloat16
    f16 = mybir.dt.float16
    AF = mybir.ActivationFunctionType

    nc = bacc.Bacc("TRN2", target_bir_lowering=False, debug=False,
                   num_devices=num_cores, num_swdge_queues=4)

    CT = call_cols[-1][-1][2]
    n_groups = len(call_cols)
    NTC = -(-NT // CGT)
    Cg_max = max(calls[-1][2] - calls[0][1] for calls in call_cols)
    S_max = max(sum(hi - lo for lo, hi in rr) for rr in tile_ranges)

    # ---- external I/O ----
    embT_d = nc.dram_tensor("embT", [NF, EMB, V], f32, kind="ExternalInput")
    w1a_d = nc.dram_tensor("w1a", [D, D], f32, kind="ExternalInput")
    w1b_d = nc.dram_tensor("w1b", [D, D], f32, kind="ExternalInput")
    w2a_d = nc.dram_tensor("w2a", [D, D], f32, kind="ExternalInput")
    w2b_d = nc.dram_tensor("w2b", [D, D], f32, kind="ExternalInput")
    cmeta_d = nc.dram_tensor("cmeta16", [128, NT * NF * P // 16], i16,
                             kind="ExternalInput")
    idx_d = nc.dram_tensor("idx_meta", [128, CT * P // 16], i16,
                           kind="ExternalInput")
    drel_d = nc.dram_tensor("drel_meta", [P, CT], f16, kind="ExternalInput")
    iota_d = nc.dram_tensor("iota_row", [GSZ, P, P], f16, kind="ExternalInput")
    ident_d = nc.dram_tensor("identity", [P, P], f32, kind="ExternalInput")
    bias_d = None
    if use_biases or use_ln_gb:
        # rows: b1a, b1b, b2a, b2b, ln1_g, ln1_b, ln2_g, ln2_b
        bias_d = nc.dram_tensor("biasrows", [8, D], f32, kind="ExternalInput")
    out_d = nc.dram_tensor("out", [NSH, D], f32, kind="ExternalOutput")

    groups = [list(range(num_cores))]

    from contextlib import ExitStack

    qctr = [0]
    emit_names = []

    def next_q():
        i = qctr[0]
        qctr[0] += 1
        return queue_map[i] if queue_map is not None else 0

    with tile.TileContext(nc) as tc, ExitStack() as ctx:
        singles = ctx.enter_context(tc.tile_pool(name="singles", bufs=1))
        dram = ctx.enter_context(tc.tile_pool(name="dram", bufs=1, space="DRAM"))
        meta_p = ctx.enter_context(tc.tile_pool(name="meta", bufs=4))
        gath_p = ctx.enter_context(tc.tile_pool(name="gath", bufs=2))
        g4_p = ctx.enter_context(tc.tile_pool(name="g4", bufs=2))
        sel_p = ctx.enter_context(tc.tile_pool(name="sel", bufs=3))
        work_p = ctx.enter_context(tc.tile_pool(name="work", bufs=4))
        stat_p = ctx.enter_context(tc.tile_pool(name="stat", bufs=4))
        ps_agg = ctx.enter_context(tc.tile_pool(name="ps_agg", bufs=3, space="PSUM"))
        ps_tr = ctx.enter_context(tc.tile_pool(name="ps_tr", bufs=2, space="PSUM"))
        ps_mm = ctx.enter_context(tc.tile_pool(name="ps_mm", bufs=2, space="PSUM"))

        # ---- internal DRAM tables ----
        t_dram = dram.tile([NF * V, D], bf16)
        z_shard = dram.tile([NSH, D], bf16)
        z2_shard = dram.tile([NSH, D], bf16)
        z_full = [dram.tile([nr * num_cores, D], bf16, addr_space="Shared",
                            name=f"z_full_{s}")
                  for s, (rb, nr) in enumerate(SL_ROWS)]
        z2_full = [dram.tile([nr * num_cores, D], bf16, addr_space="Shared",
                             name=f"z2_full_{s}")
                   for s, (rb, nr) in enumerate(SL_ROWS)]

        # ---- persistent SBUF constants ----
        iota_sb = singles.tile([P, GSZ, P], f16)
        for tt in range(GSZ):
            nc.sync.dma_start(out=iota_sb[:, tt, :], in_=iota_d[tt])
        ident_sb = singles.tile([P, P], f32)
        nc.sync.dma_start(out=ident_sb[:], in_=ident_d[:])
        eps_sb = singles.tile([P, 1], f32)
        nc.vector.memset(eps_sb[:], LN_EPS)

        def load_w(dram_t, name):
            tiles = []
            for kk in range(2):
                w_sb = singles.tile([P, D], f32, name=f"{name}_{kk}")
                nc.sync.dma_start(out=w_sb[:], in_=dram_t[kk * P:(kk + 1) * P, :])
                tiles.append(w_sb)
            return tiles

        w1a_rows = []
        for f in range(NF):
            w1a_r = singles.tile([EMB, D], f32, name=f"w1a_r{f}")
            nc.sync.dma_start(out=w1a_r[:], in_=w1a_d[f * EMB:(f + 1) * EMB, :])
            w1a_rows.append(w1a_r)

        def round_w(tiles, name):
            out = []
            for kk, w_sb in enumerate(tiles):
                w_r = singles.tile([P, D], bf16, name=f"{name}r_{kk}")
                nc.vector.tensor_copy(out=w_r[:], in_=w_sb[:])
                out.append(w_r)
            return out

        w1b_sb = round_w(load_w(w1b_d, "w1b"), "w1b")
        w2a_sb = round_w(load_w(w2a_d, "w2a"), "w2a")
        w2b_sb = round_w(load_w(w2b_d, "w2b"), "w2b")

        bias_sb = None
        if bias_d is not None:
            bias_tile = singles.tile([P, 8, D], f32)
            for r in range(8):
                nc.sync.dma_start(
                    out=bias_tile[:, r, :],
                    in_=bias_d[r].unsqueeze(0).to_broadcast([P, D]))
            bias_sb = [bias_tile[:, r, :] for r in range(8)]

        # NaN guard: skipped (-1) gather slots keep stale SBUF bytes that
        # feed the aggregation matmul multiplied by S==0; 0 * NaN would
        # poison PSUM, so zero the two G ring buffers once.
        for _ in range(2):
            Gz = gath_p.tile([P, Cg_max, D], bf16, tag="G")
            nc.vector.memset(Gz[:], 0.0)

        # =================================================================
        # Phase B: t tables  t[f] = emb_f @ w1a[64f:64f+64, :]   -> t_dram
        # =================================================================
        MT = 125  # 1000 = 8 * 125
        embT_p = ctx.enter_context(tc.tile_pool(name="embT_p", bufs=1))
        for f in range(NF):
            embT_sb = embT_p.tile([EMB, V], f32, tag="embT")
            nc.sync.dma_start(out=embT_sb[:], in_=embT_d[f])
            w_rows = w1a_rows[f][:]
            for j in range(V // MT):
                t_ps = ps_mm.tile([MT, D], f32, tag="tps", bufs=1)
                nc.tensor.matmul(out=t_ps[:],
                                 lhsT=embT_sb[:, j * MT:(j + 1) * MT],
                                 rhs=w_rows, start=True, stop=True)
                t_sb = work_p.tile([MT, D], bf16, tag="tsb")
                nc.vector.tensor_copy(out=t_sb[:], in_=t_ps[:])
                nc.sync.dma_start(
                    out=t_dram[f * V + j * MT:f * V + (j + 1) * MT, :],
                    in_=t_sb[:])

        # =================================================================
        # Phase C: z shard  z[n] = sum_f t[cmeta[n, f]]          -> z_shard
        # (sliced AllGather overlaps the build)
        # =================================================================
        sl_of_tile = {}
        for s, (rbase, nr) in enumerate(SL_ROWS):
            sl_of_tile[(rbase + nr - 1) // P] = s

        for gg in range(NTC):
            t0 = gg * CGT
            ntl = min(CGT, NT - t0)
            L = ntl * NF * P
            cm = meta_p.tile([128, CGT * NF * P // 16], i16, tag="cm")
            o16 = t0 * NF * P // 16
            nc.sync.dma_start(out=cm[:, :L // 16],
                              in_=cmeta_d[:, o16:o16 + L // 16])
            g4 = g4_p.tile([P, CGT * NF, D], bf16, tag="g4")
            for s in range(0, ntl * NF, 8):
                e = min(s + 8, ntl * NF)
                inst = nc.gpsimd.dma_gather(
                    out_ap=g4[:, s:e, :], in_ap=t_dram[:, :],
                    idxs_ap=cm[:, s * 8:e * 8],
                    num_idxs=(e - s) * P, num_idxs_reg=(e - s) * P,
                    elem_size=D, queue_num=next_q())
                emit_names.append(inst.ins.name)
            for tt in range(ntl):
                t = t0 + tt
                valid = LAST_VALID if t == NT - 1 else P
                t01 = work_p.tile([P, 2, D], f32, tag="t01")
                nc.vector.tensor_add(out=t01[:],
                                     in0=g4[:, tt * NF:tt * NF + 2, :],
                                     in1=g4[:, tt * NF + 2:tt * NF + 4, :])
                z_t = work_p.tile([P, D], bf16, tag="z_t")
                nc.vector.tensor_add(out=z_t[:], in0=t01[:, 0, :],
                                     in1=t01[:, 1, :])
                nc.sync.dma_start(out=z_shard[t * P:t * P + valid, :],
                                  in_=z_t[:valid, :])
                if t in sl_of_tile:
                    s = sl_of_tile[t]
                    rbase, nr = SL_ROWS[s]
                    nc.gpsimd.collective_compute(
                        "AllGather", mybir.AluOpType.bypass,
                        replica_groups=groups,
                        ins=[z_shard[rbase:rbase + nr, :]],
                        outs=[z_full[s][:]])

        # =================================================================
        # Phases D/E: message passing + MLP layers
        # =================================================================
        def mp_layer(layer):
            tab_full = z_full if layer == 1 else z2_full
            tab_own = z_shard if layer == 1 else z2_shard
            wb_sb = w1b_sb if layer == 1 else w2b_sb
            ba_row, bb_row = (0, 1) if layer == 1 else (2, 3)
            g_row, b_row = (4, 5) if layer == 1 else (6, 7)

            for g in range(n_groups):
                calls = call_cols[g]
                g_lo = calls[0][1]
                g_hi = calls[-1][2]
                Cg = g_hi - g_lo
                if Cg == 0:
                    continue
                # --- metadata ---
                idx_sb = meta_p.tile([128, Cg_max * 8], i16, tag="idx")
                nc.sync.dma_start(
                    out=idx_sb[:, :Cg * 8],
                    in_=idx_d[:, g_lo * 8:g_hi * 8])
                drel = meta_p.tile([P, Cg_max], f16, tag="drel")
                nc.sync.dma_start(out=drel[:, :Cg],
                                  in_=drel_d[:, g_lo:g_hi])
                # --- gather: <=8-chunk calls, one bucket each ---
                G = gath_p.tile([P, Cg_max, D], bf16, tag="G")
                for b, c_lo, c_hi, nv in calls:
                    nb = c_hi - c_lo
                    if nb == 0 or nv == 0:
                        continue
                    inst = nc.gpsimd.dma_gather(
                        out_ap=G[:, c_lo - g_lo:c_hi - g_lo, :],
                        idxs_ap=idx_sb[:, (c_lo - g_lo) * 8:(c_hi - g_lo) * 8],
                        in_ap=tab_full[b][:, :],
                        num_idxs=nb * P, num_idxs_reg=nv, elem_size=D,
                        queue_num=next_q())
                    emit_names.append(inst.ins.name)
                for t in range(g * GSZ, min((g + 1) * GSZ, NT)):
                    tt = t - g * GSZ
                    valid = LAST_VALID if t == NT - 1 else P
                    ranges = tile_ranges[t]
                    ncols = sum(hi - lo for lo, hi in ranges)
                    # --- tile one-hot: S[e, ci, d] = (drel_enc == d+128*tt),
                    # built per bucket range against the shifted iota ---
                    S = sel_p.tile([P, S_max, P], bf16, tag="S")
                    ci0 = 0
                    for lo, hi in ranges:
                        w = hi - lo
                        nc.vector.tensor_tensor(
                            out=S[:, ci0:ci0 + w, :],
                            in0=drel[:, lo - g_lo:hi - g_lo].unsqueeze(2)
                                .to_broadcast([P, w, P]),
                            in1=iota_sb[:, tt, :].unsqueeze(1)
                                .to_broadcast([P, w, P]),
                            op=mybir.AluOpType.is_equal)
                        ci0 += w
                    # --- aggregate: agg[d, :] += S[:, c, d].T @ G[:, c, :] ---
                    agg_ps = ps_agg.tile([P, D], f32, tag="agg")
                    ci = 0
                    for lo, hi in ranges:
                        for c in range(lo - g_lo, hi - g_lo):
                            nc.tensor.matmul(out=agg_ps[:],
                                             lhsT=S[:, ci, :], rhs=G[:, c, :],
                                             start=(ci == 0),
                                             stop=(ci == ncols - 1))
                            ci += 1
                    # --- u = relu(z_own + agg (+ba)) ---
                    zown = work_p.tile([P, D], bf16, tag="zown")
                    if valid < P:
                        nc.vector.memset(zown[:], 0.0)
                    nc.sync.dma_start(out=zown[:valid, :],
                                      in_=tab_own[t * P:t * P + valid, :])
                    u = work_p.tile([P, D], f32, tag="u")
                    nc.vector.tensor_add(out=u[:], in0=agg_ps[:], in1=zown[:])
                    if use_biases:
                        nc.vector.tensor_add(out=u[:], in0=u[:],
                                             in1=bias_sb[ba_row])
                    ur = work_p.tile([P, D], f32, tag="ur")
                    nc.scalar.activation(out=ur[:], in_=u[:], func=AF.Relu)
                    # --- v = u @ wb (+bb) ---
                    uT_ps = ps_tr.tile([P, 2, P], f32, tag="uT_ps")
                    for kk in range(2):
                        nc.tensor.transpose(out=uT_ps[:, kk, :],
                                            in_=ur[:, kk * P:(kk + 1) * P],
                                            identity=ident_sb[:])
                    uT = work_p.tile([P, 2, P], bf16, tag="uT")
                    nc.scalar.activation(out=uT[:], in_=uT_ps[:], func=AF.Copy)
                    v_ps = ps_mm.tile([P, D], f32, tag="v_ps")
                    for kk in range(2):
                        nc.tensor.matmul(out=v_ps[:],
                                         lhsT=uT[:, kk, :], rhs=wb_sb[kk][:],
                                         start=(kk == 0), stop=(kk == 1))
                    r = work_p.tile([P, D], f32, tag="r")
                    if use_biases:
                        nc.vector.tensor_add(out=r[:], in0=v_ps[:],
                                             in1=bias_sb[bb_row])
                        if layer == 1:
                            nc.vector.tensor_scalar_max(out=r[:], in0=r[:],
                                                        scalar1=0.0)
                    else:
                        nc.scalar.activation(
                            out=r[:], in_=v_ps[:],
                            func=AF.Relu if layer == 1 else AF.Copy)
                    # --- LayerNorm ---
                    stats = stat_p.tile([P, 6], f32, tag="stats")
                    nc.vector.bn_stats(out=stats[:], in_=r[:])
                    mv = stat_p.tile([P, 2], f32, tag="mv")
                    nc.vector.bn_aggr(out=mv[:], in_=stats[:])
                    nc.scalar.activation(out=mv[:, 1:2], in_=mv[:, 1:2],
                                         func=AF.Sqrt,
                                         bias=eps_sb[:], scale=1.0)
                    nc.vector.reciprocal(out=mv[:, 1:2], in_=mv[:, 1:2])
                    nm = stat_p.tile([P, 1], f32, tag="nm")
                    nc.vector.tensor_scalar(out=nm[:], in0=mv[:, 0:1],
                                            scalar1=mv[:, 1:2], scalar2=-1.0,
                                            op0=mybir.AluOpType.mult,
                                            op1=mybir.AluOpType.mult)
                    h = work_p.tile([P, D], f32, tag="h")
                    nc.scalar.activation(out=h[:], in_=r[:], func=AF.Identity,
                                         bias=nm[:], scale=mv[:, 1:2])
                    if use_ln_gb:
                        nc.vector.tensor_mul(out=h[:], in0=h[:],
                                             in1=bias_sb[g_row])
                        nc.vector.tensor_add(out=h[:], in0=h[:],
                                             in1=bias_sb[b_row])

                    if layer == 1:
                        # --- z2 = h @ w2a -> z2_shard ---
                        hT_ps = ps_tr.tile([P, 2, P], f32, tag="uT_ps")
                        for kk in range(2):
                            nc.tensor.transpose(out=hT_ps[:, kk, :],
                                                in_=h[:, kk * P:(kk + 1) * P],
                                                identity=ident_sb[:])
                        hT = work_p.tile([P, 2, P], bf16, tag="uT")
                        nc.scalar.activation(out=hT[:], in_=hT_ps[:],
                                             func=AF.Copy)
                        z2_ps = ps_mm.tile([P, D], f32, tag="v_ps")
                        for kk in range(2):
                            nc.tensor.matmul(out=z2_ps[:],
                                             lhsT=hT[:, kk, :],
                                             rhs=w2a_sb[kk][:],
                                             start=(kk == 0), stop=(kk == 1))
                        z2_sb = work_p.tile([P, D], bf16, tag="z2_sb")
                        nc.scalar.activation(out=z2_sb[:], in_=z2_ps[:],
                                             func=AF.Copy)
                        nc.sync.dma_start(
                            out=z2_shard[t * P:t * P + valid, :],
                            in_=z2_sb[:valid, :])
                        if t in sl_of_tile:
                            s = sl_of_tile[t]
                            rbase, nr = SL_ROWS[s]
                            nc.gpsimd.collective_compute(
                                "AllGather", mybir.AluOpType.bypass,
                                replica_groups=groups,
                                ins=[z2_shard[rbase:rbase + nr, :]],
                                outs=[z2_full[s][:]])
                    else:
                        nc.sync.dma_start(out=out_d[t * P:t * P + valid, :],
                                          in_=h[:valid, :])

        mp_layer(1)
        mp_layer(2)

    from concourse.tile_sem_assignment import DMAInst
    final_names = []
    for fn in nc.m.functions:
        for blk in fn.blocks:
            for inst in blk.instructions:
                if (isinstance(inst, DMAInst)
                        and inst.engine == mybir.EngineType.Pool):
                    final_names.append(inst.name)
    emit_idx = {nm: i for i, nm in enumerate(emit_names)}
    qmap = [0] * len(emit_names)
    aligned = True
    for pos, nm in enumerate(final_names):
        qmap[emit_idx[nm]] = pos % 4
        if queue_map is not None and queue_map[emit_idx[nm]] != pos % 4:
            aligned = False
    if compile_now:
        nc.compile()
    return nc, qmap, aligned


def get_program(Cb, call_cols, tile_ranges, **kw):
    key = (tuple(tuple(c) for c in Cb),
           tuple(tuple(c) for cs in call_cols for c in cs),
           tuple(sorted(kw.items())))
    if key not in _PROGRAM_CACHE:
        _, qmap, _ = _build_program(Cb, call_cols, tile_ranges,
                                    compile_now=False, **kw)
        for _ in range(3):
            nc, qmap2, aligned = _build_program(
                Cb, call_cols, tile_ranges, queue_map=qmap,
                compile_now=True, **kw)
            if aligned:
                break
            qmap = qmap2
        assert aligned, "SWDGE queue assignment did not converge"
        _PROGRAM_CACHE[key] = nc
    return _PROGRAM_CACHE[key]


# ---------------------------------------------------------------------------
# Entry point
# ---------------------------------------------------------------------------
def kernel_with_results(x_cat, edge_index, emb0, emb1, emb2, emb3,
                        w1a, b1a, w1b, b1b, w2a, b2a, w2b, b2b,
                        ln1_g, ln1_b, ln2_g, ln2_b, trace=False):
    import ml_dtypes
    from concourse import bass_utils

    Cb, call_cols, tile_ranges, cmeta16, idx_meta, drel_meta = _prep_meta(
        x_cat, edge_index)

    f32 = np.float32
    embT = np.stack([np.ascontiguousarray(np.asarray(e, f32).T)
                     for e in (emb0, emb1, emb2, emb3)])
    w1a = np.ascontiguousarray(np.asarray(w1a, f32))
    w1b = np.ascontiguousarray(np.asarray(w1b, f32))
    w2a = np.ascontiguousarray(np.asarray(w2a, f32))
    w2b = np.ascontiguousarray(np.asarray(w2b, f32))

    biases = [np.asarray(b, f32) for b in (b1a, b1b, b2a, b2b)]
    lngb = [np.asarray(b, f32) for b in (ln1_g, ln1_b, ln2_g, ln2_b)]
    use_biases = any(np.any(b != 0.0) for b in biases)
    use_ln_gb = (np.any(lngb[0] != 1.0) or np.any(lngb[1] != 0.0)
                 or np.any(lngb[2] != 1.0) or np.any(lngb[3] != 0.0))

    iota_row = np.stack([
        np.broadcast_to((np.arange(P) + P * tt).astype(np.float16), (P, P))
        for tt in range(GSZ)]).copy()
    identity = np.eye(P, dtype=f32)

    nc = get_program(Cb, call_cols, tile_ranges, use_biases=use_biases,
                     use_ln_gb=use_ln_gb)

    in_maps = []
    for k in range(NC):
        m = {
            "embT": embT,
            "w1a": w1a, "w1b": w1b, "w2a": w2a, "w2b": w2b,
            "cmeta16": cmeta16[k],
            "idx_meta": idx_meta[k],
            "drel_meta": drel_meta[k],
            "iota_row": iota_row,
            "identity": identity,
        }
        if use_biases or use_ln_gb:
            m["biasrows"] = np.stack(biases + lngb)
        in_maps.append(m)

    res = bass_utils.run_bass_kernel_spmd(nc, in_maps, core_ids=list(range(NC)),
                                          trace=trace)
    out = np.concatenate([r["out"] for r in res.results], axis=0)
    return out.astype(np.float32), res


def kernel(**inputs):
    out, _ = kernel_with_results(**inputs)
    return out


# revision 18
# speedup vs baseline: 2.7485x; 1.0072x over previous
"""Trainium2 Bass kernel for a 2-layer categorical GIN encoder.

Graph: N=100000 nodes, E=1600000 edges, 256-dim features.

    x   = concat_i emb_i[x_cat[:, i]]                  # [N, 256]
    h1  = LN1(relu(relu((x + A x) @ w1a + b1a) @ w1b + b1b))
    out = LN2(relu((h1 + A h1) @ w2a + b2a) @ w2b + b2b)

where (A x)[d] = sum over edges (s -> d) of x[s].

Strategy (8 NeuronCores, SPMD):
  * Linearity trick: (x + Ax) @ w1a == z + A z with z = x @ w1a, and
    z[n] = sum_i t_i[x_cat[n, i]] where t_i = emb_i @ w1a[64i:64i+64, :].
    t tables are built on device (bf16); each core builds its z shard by
    dma_gather of t rows; shards are AllGathered in 4 row slices
    (overlapped with the z build) into a slice-major full table
    (global row = (slice, core, row)) so collective in/outs stay
    contiguous.
  * Edges are sorted by destination and sharded by destination range
    (12500 nodes per core) -> the aggregation needs no collective.
  * Per-edge gathers of z[src] use the InstDMAGatherAnt ucode whose
    ~1us SWDGE overhead is per *call*: calls cover up to 8 chunks
    (1024 edges, the SWDGE descriptor-ring cap) of a 4-tile group,
    one bucket (25000 permuted rows, int16 index limit) at a time,
    rotating over 4 SWDGE queues.  Edges are src-sorted inside each
    (tile, bucket) segment so descriptors read ascending addresses.
    Slots beyond a segment's max-over-cores edge count carry idx -1
    (skipped by the ucode, zero HBM traffic); slots between this
    core's count and the max carry idx 0 (row 0 fetch) so the valid
    count baked into the SPMD program is core-uniform.
  * Aggregation per tile: one-hot S[e, d] = (dst_rel[e] == d) built on
    DVE (bf16), agg += S.T @ G in PSUM (bf16 in, fp32 accumulate);
    padding slots have dst_rel -1 and select zero.
  * MLP per tile: PE transposes + bf16 weight matmuls; relu / LayerNorm
    application / PSUM->SBUF casts run on the Activation engine
    (func(scale*x+bias) with per-partition scale/bias) to unload DVE;
    bn_stats/bn_aggr remain on DVE.  Layer 1 also applies w2a so the
    second (also sliced) AllGather ships z2 = h1 @ w2a.
"""

import numpy as np

# ---------------------------------------------------------------------------
# Problem constants (hardcoded per contest contract)
# ---------------------------------------------------------------------------
N = 100000        # nodes
E = 1600000       # edges
D = 256           # feature dim (in = hidden = out)
EMB = 64          # per-field embedding dim
V = 1000          # categories per field
NF = 4            # categorical fields
NC = 8            # NeuronCores
P = 128           # partitions
LN_EPS = 1e-5

NSH = N // NC             # nodes per core (12500)
NT = (NSH + P - 1) // P   # node tiles per core (98)
LAST_VALID = NSH - (NT - 1) * P  # valid rows in last tile (84)

NBUK = 4                  # source buckets == allgather slices
GSZ = 4                   # dst tiles per gather group
CGT = 4                   # tiles per phase-C gather call
MAXC = 8                  # chunks per dma_gather call (1024-desc ring cap)
NSL = 4                   # allgather slices

# slice layout: tiles per slice -> local row ranges
_SL_TILES = [25, 25, 25, 23]
SL_ROWS = []              # (row_base, nrows) per slice
_rb = 0
for _s in range(NSL):
    _nr = min(_SL_TILES[_s] * P, NSH - _rb)
    SL_ROWS.append((_rb, _nr))
    _rb += _nr
assert _rb == NSH
GLOB_OFF = [0]
for _rb, _nr in SL_ROWS:
    GLOB_OFF.append(GLOB_OFF[-1] + _nr * NC)


def _permute_rows(node):
    """node id -> slice-major global row (slice, core, local row)."""
    k = node // NSH
    r = node - k * NSH
    out = np.zeros_like(node)
    for s, (rb, nr) in enumerate(SL_ROWS):
        m = (r >= rb) & (r < rb + nr)
        out = np.where(m, GLOB_OFF[s] + k * nr + (r - rb), out)
    return out


def _wrap_idx(flat):
    """[L] int16 (L % 16 == 0) -> [128, L//16] wrapped + replicated."""
    w = flat.reshape(-1, 16).T.copy()           # [16, L//16]
    return np.tile(w, (8, 1))                   # [128, L//16]


# ---------------------------------------------------------------------------
# Host-side preprocessing: shard + sort edges, build per-core metadata
# ---------------------------------------------------------------------------
def _prep_meta(x_cat, edge_index):
    import ml_dtypes

    src = np.asarray(edge_index[0], dtype=np.int64)
    dst = np.asarray(edge_index[1], dtype=np.int64)
    prow = _permute_rows(src)

    order = np.argsort(dst, kind="stable")
    dst_s = dst[order]
    prow_s = prow[order]
    bounds = np.searchsorted(dst_s, np.arange(NC + 1) * NSH)

    n_groups = -(-NT // GSZ)
    per_core = []
    counts_tb = np.zeros((NC, NT * NBUK), dtype=np.int64)
    for k in range(NC):
        lo, hi = bounds[k], bounds[k + 1]
        d_k = dst_s[lo:hi] - k * NSH
        p_k = prow_s[lo:hi]
        t_k = d_k // P
        rel = (d_k - t_k * P).astype(np.int64)
        b_k = np.searchsorted(GLOB_OFF, p_k, side="right") - 1
        g_k = t_k // GSZ
        # region key: (group, bucket); order inside: (tile, src row)
        key = t_k * NBUK + b_k
        rkey = (g_k * NBUK + b_k) * (GSZ + 1) + (t_k - g_k * GSZ)
        o2 = np.lexsort((p_k, rkey))
        key = key[o2]
        counts = np.bincount(key, minlength=NT * NBUK)
        counts_tb[k] = counts
        per_core.append((p_k[o2], rel[o2], (t_k - g_k * GSZ)[o2],
                         (g_k * NBUK + b_k)[o2], key))

    # per-(group,bucket) region sizes: max total edges over cores
    gb_count = np.zeros((NC, n_groups * NBUK), dtype=np.int64)
    for k in range(NC):
        c = counts_tb[k].reshape(NT, NBUK)
        for g in range(n_groups):
            gb_count[k, g * NBUK:(g + 1) * NBUK] = (
                c[g * GSZ:(g + 1) * GSZ].sum(axis=0))
    gb_max = gb_count.max(axis=0)                     # [n_groups*NBUK]
    gb_chunks = -(-gb_max // P)

    # per-(tile,bucket) start offsets inside the region, per core, plus
    # the union chunk range over cores for the tile matmuls
    tb_lo = np.full((NT, NBUK), 1 << 60, dtype=np.int64)
    tb_hi = np.zeros((NT, NBUK), dtype=np.int64)
    core_tb_start = np.zeros((NC, NT, NBUK), dtype=np.int64)
    for k in range(NC):
        c = counts_tb[k].reshape(NT, NBUK)
        for g in range(n_groups):
            tiles = list(range(g * GSZ, min((g + 1) * GSZ, NT)))
            for b in range(NBUK):
                off = 0
                for t in tiles:
                    core_tb_start[k, t, b] = off
                    lo_ch = off // P
                    off += c[t, b]
                    hi_ch = -(-off // P) if c[t, b] > 0 else lo_ch
                    tb_lo[t, b] = min(tb_lo[t, b], lo_ch)
                    tb_hi[t, b] = max(tb_hi[t, b], hi_ch)

    # global chunk columns: regions in (group, bucket) order
    reg_col = np.zeros(n_groups * NBUK, dtype=np.int64)
    cc = 0
    call_cols = []           # per group: [(b, col_lo, col_hi, n_valid)]
    for g in range(n_groups):
        calls = []
        for b in range(NBUK):
            gi = g * NBUK + b
            reg_col[gi] = cc
            nch = int(gb_chunks[gi])
            nv_total = int(gb_max[gi])
            for s in range(0, nch, MAXC):
                e = min(s + MAXC, nch)
                nv = max(0, min(nv_total - s * P, (e - s) * P))
                if nv > 0:
                    calls.append((b, int(cc + s), int(cc + e), int(nv)))
            cc += nch
        call_cols.append(calls)
    CT = int(cc)

    # per-tile chunk ranges (union over cores) + in-group tile index
    tile_ranges = []
    for t in range(NT):
        g = t // GSZ
        rr = []
        for b in range(NBUK):
            gi = g * NBUK + b
            if tb_hi[t, b] > tb_lo[t, b]:
                rr.append((int(reg_col[gi] + tb_lo[t, b]),
                           int(reg_col[gi] + tb_hi[t, b])))
        tile_ranges.append(rr)

    idx_meta, drel_meta = [], []
    for k in range(NC):
        p_k, rel_k, tin_k, gi_k, key_k = per_core[k]
        # rank within each (t,b) run of the region-sorted order
        run_first = np.zeros(NT * NBUK, dtype=np.int64)
        rs = np.concatenate([[0], np.flatnonzero(np.diff(key_k) != 0) + 1])
        run_first[key_k[rs]] = rs
        j_seg = np.arange(len(p_k)) - run_first[key_k]
        t_all = key_k // NBUK
        b_all = key_k % NBUK
        off_in_reg = core_tb_start[k, t_all, b_all] + j_seg
        slot = reg_col[gi_k] * P + off_in_reg           # compact flat slot

        idxflat = np.full(CT * P, -1, dtype=np.int16)
        drelflat = np.full(CT * P, -1.0, dtype=np.float32)
        # pad [n_core, n_max) of each region with idx 0 (uniform reg)
        for gi in range(n_groups * NBUK):
            n_k = int(gb_count[k, gi])
            n_mx = int(gb_max[gi])
            if n_mx > n_k:
                base = reg_col[gi] * P
                idxflat[base + n_k:base + n_mx] = 0
        idxflat[slot] = (p_k - np.asarray(GLOB_OFF)[b_all]).astype(np.int16)
        drelflat[slot] = rel_k + P * tin_k              # tile-in-group coded

        idx_meta.append(_wrap_idx(idxflat))
        drel_meta.append(
            drelflat.reshape(CT, P).T.astype(np.float16).copy())

    # phase-C index stream: slot (tile_in_call*NF + f)*128 + p
    x_cat = np.asarray(x_cat, dtype=np.int64)
    cmeta16 = []
    for k in range(NC):
        xc = x_cat[k * NSH:(k + 1) * NSH]               # [NSH, NF]
        rows = (xc + np.arange(NF)[None, :] * V).astype(np.int16)
        cm = np.zeros((NT, NF, P), dtype=np.int16)
        for t in range(NT):
            v = min(P, NSH - t * P)
            cm[t, :, :v] = rows[t * P:t * P + v].T
        cmeta16.append(_wrap_idx(cm.reshape(-1)))

    Cb_list = [[int(x) for x in row] for row in gb_count.max(axis=0).reshape(
        n_groups, NBUK)]
    return Cb_list, call_cols, tile_ranges, cmeta16, idx_meta, drel_meta


# ---------------------------------------------------------------------------
# Device program
# ---------------------------------------------------------------------------
_PROGRAM_CACHE = {}


def _build_program(Cb, call_cols, tile_ranges,
                   use_biases=False, use_ln_gb=False, num_cores=NC,
                   queue_map=None, compile_now=True):
    """Build (+ optionally compile) the SPMD Bass program.

    queue_map: emission-index -> SWDGE queue for the gather calls.  The
    scheduler may reorder gathers (e.g. hoist layer-2 bucket-b gathers
    that only need z2 slice b), and each DMASW semaphore (assigned
    final-order mod 8) must stay on one queue, so queues are assigned
    from the *final* order learned in a first pass.
    """
    import concourse.bacc as bacc
    import concourse.bass as bass
    import concourse.tile as tile
    from concourse import mybir

    f32 = mybir.dt.float32
    i16 = mybir.dt.int16
    bf16 = mybir.dt.bfloat16
    f16 = mybir.dt.float16
    AF = mybir.ActivationFunctionType

    nc = bacc.Bacc("TRN2", target_bir_lowering=False, debug=False,
                   num_devices=num_cores, num_swdge_queues=4)

    CT = call_cols[-1][-1][2]
    n_groups = len(call_cols)
    NTC = -(-NT // CGT)
    Cg_max = max(calls[-1][2] - calls[0][1] for calls in call_cols)
    S_max = max(sum(hi - lo for lo, hi in rr) for rr in tile_ranges)

    # ---- external I/O ----
    embT_d = nc.dram_tensor("embT", [NF, EMB, V], f32, kind="ExternalInput")
    w1a_d = nc.dram_tensor("w1a", [D, D], f32, kind="ExternalInput")
    w1b_d = nc.dram_tensor("w1b", [D, D], f32, kind="ExternalInput")
    w2a_d = nc.dram_tensor("w2a", [D, D], f32, kind="ExternalInput")
    w2b_d = nc.dram_tensor("w2b", [D, D], f32, kind="ExternalInput")
    cmeta_d = nc.dram_tensor("cmeta16", [128, NT * NF * P // 16], i16,
                             kind="ExternalInput")
    idx_d = nc.dram_tensor("idx_meta", [128, CT * P // 16], i16,
                           kind="ExternalInput")
    drel_d = nc.dram_tensor("drel_meta", [P, CT], f16, kind="ExternalInput")
    iota_d = nc.dram_tensor("iota_row", [GSZ, P, P], f16, kind="ExternalInput")
    ident_d = nc.dram_tensor("identity", [P, P], f32, kind="ExternalInput")
    bias_d = None
    if use_biases or use_ln_gb:
        # rows: b1a, b1b, b2a, b2b, ln1_g, ln1_b, ln2_g, ln2_b
        bias_d = nc.dram_tensor("biasrows", [8, D], f32, kind="ExternalInput")
    out_d = nc.dram_tensor("out", [NSH, D], f32, kind="ExternalOutput")

    groups = [list(range(num_cores))]

    from contextlib import ExitStack

    qctr = [0]
    emit_names = []

    def next_q():
        i = qctr[0]
        qctr[0] += 1
        return queue_map[i] if queue_map is not None else 0

    with tile.TileContext(nc) as tc, ExitStack() as ctx:
        singles = ctx.enter_context(tc.tile_pool(name="singles", bufs=1))
        dram = ctx.enter_context(tc.tile_pool(name="dram", bufs=1, space="DRAM"))
        meta_p = ctx.enter_context(tc.tile_pool(name="meta", bufs=4))
        gath_p = ctx.enter_context(tc.tile_pool(name="gath", bufs=2))
        g4_p = ctx.enter_context(tc.tile_pool(name="g4", bufs=3))
        sel_p = ctx.enter_context(tc.tile_pool(name="sel", bufs=3))
        work_p = ctx.enter_context(tc.tile_pool(name="work", bufs=4))
        stat_p = ctx.enter_context(tc.tile_pool(name="stat", bufs=4))
        ps_agg = ctx.enter_context(tc.tile_pool(name="ps_agg", bufs=3, space="PSUM"))
        ps_tr = ctx.enter_context(tc.tile_pool(name="ps_tr", bufs=2, space="PSUM"))
        ps_mm = ctx.enter_context(tc.tile_pool(name="ps_mm", bufs=2, space="PSUM"))

        # ---- internal DRAM tables ----
        t_dram = dram.tile([NF * V, D], bf16)
        z_shard = dram.tile([NSH, D], bf16)
        z2_shard = dram.tile([NSH, D], bf16)
        z_full = [dram.tile([nr * num_cores, D], bf16, addr_space="Shared",
                            name=f"z_full_{s}")
                  for s, (rb, nr) in enumerate(SL_ROWS)]
        z2_full = [dram.tile([nr * num_cores, D], bf16, addr_space="Shared",
                             name=f"z2_full_{s}")
                   for s, (rb, nr) in enumerate(SL_ROWS)]

        # ---- persistent SBUF constants ----
        iota_sb = singles.tile([P, GSZ, P], f16)
        for tt in range(GSZ):
            nc.sync.dma_start(out=iota_sb[:, tt, :], in_=iota_d[tt])
        ident_sb = singles.tile([P, P], f32)
        nc.sync.dma_start(out=ident_sb[:], in_=ident_d[:])
        eps_sb = singles.tile([P, 1], f32)
        nc.vector.memset(eps_sb[:], LN_EPS)

        def load_w(dram_t, name):
            tiles = []
            for kk in range(2):
                w_sb = singles.tile([P, D], f32, name=f"{name}_{kk}")
                nc.sync.dma_start(out=w_sb[:], in_=dram_t[kk * P:(kk + 1) * P, :])
                tiles.append(w_sb)
            return tiles

        w1a_rows = []
        for f in range(NF):
            w1a_r = singles.tile([EMB, D], f32, name=f"w1a_r{f}")
            nc.sync.dma_start(out=w1a_r[:], in_=w1a_d[f * EMB:(f + 1) * EMB, :])
            w1a_rows.append(w1a_r)

        def round_w(tiles, name):
            out = []
            for kk, w_sb in enumerate(tiles):
                w_r = singles.tile([P, D], bf16, name=f"{name}r_{kk}")
                nc.vector.tensor_copy(out=w_r[:], in_=w_sb[:])
                out.append(w_r)
            return out

        w1b_sb = round_w(load_w(w1b_d, "w1b"), "w1b")
        w2a_sb = round_w(load_w(w2a_d, "w2a"), "w2a")
        w2b_sb = round_w(load_w(w2b_d, "w2b"), "w2b")

        bias_sb = None
        if bias_d is not None:
            bias_tile = singles.tile([P, 8, D], f32)
            for r in range(8):
                nc.sync.dma_start(
                    out=bias_tile[:, r, :],
                    in_=bias_d[r].unsqueeze(0).to_broadcast([P, D]))
            bias_sb = [bias_tile[:, r, :] for r in range(8)]

        # NaN guard: skipped (-1) gather slots keep stale SBUF bytes that
        # feed the aggregation matmul multiplied by S==0; 0 * NaN would
        # poison PSUM, so zero the two G ring buffers once.
        for _ in range(2):
            Gz = gath_p.tile([P, Cg_max, D], bf16, tag="G")
            nc.vector.memset(Gz[:], 0.0)

        # =================================================================
        # Phase B: t tables  t[f] = emb_f @ w1a[64f:64f+64, :]   -> t_dram
        # =================================================================
        MT = 125  # 1000 = 8 * 125
        embT_p = ctx.enter_context(tc.tile_pool(name="embT_p", bufs=1))
        for f in range(NF):
            embT_sb = embT_p.tile([EMB, V], f32, tag="embT")
            nc.sync.dma_start(out=embT_sb[:], in_=embT_d[f])
            w_rows = w1a_rows[f][:]
            for j in range(V // MT):
                t_ps = ps_mm.tile([MT, D], f32, tag="tps", bufs=1)
                nc.tensor.matmul(out=t_ps[:],
                                 lhsT=embT_sb[:, j * MT:(j + 1) * MT],
                                 rhs=w_rows, start=True, stop=True)
                t_sb = work_p.tile([MT, D], bf16, tag="tsb")
                nc.vector.tensor_copy(out=t_sb[:], in_=t_ps[:])
                nc.sync.dma_start(
                    out=t_dram[f * V + j * MT:f * V + (j + 1) * MT, :],
                    in_=t_sb[:])

        # =================================================================
        # Phase C: z shard  z[n] = sum_f t[cmeta[n, f]]          -> z_shard
        # (sliced AllGather overlaps the build)
        # =================================================================
        sl_of_tile = {}
        for s, (rbase, nr) in enumerate(SL_ROWS):
            sl_of_tile[(rbase + nr - 1) // P] = s

        for gg in range(NTC):
            t0 = gg * CGT
            ntl = min(CGT, NT - t0)
            L = ntl * NF * P
            cm = meta_p.tile([128, CGT * NF * P // 16], i16, tag="cm")
            o16 = t0 * NF * P // 16
            nc.sync.dma_start(out=cm[:, :L // 16],
                              in_=cmeta_d[:, o16:o16 + L // 16])
            g4 = g4_p.tile([P, CGT * NF, D], bf16, tag="g4")
            for s in range(0, ntl * NF, 8):
                e = min(s + 8, ntl * NF)
                inst = nc.gpsimd.dma_gather(
                    out_ap=g4[:, s:e, :], in_ap=t_dram[:, :],
                    idxs_ap=cm[:, s * 8:e * 8],
                    num_idxs=(e - s) * P, num_idxs_reg=(e - s) * P,
                    elem_size=D, queue_num=next_q())
                emit_names.append(inst.ins.name)
            for tt in range(ntl):
                t = t0 + tt
                valid = LAST_VALID if t == NT - 1 else P
                t01 = work_p.tile([P, 2, D], f32, tag="t01")
                nc.vector.tensor_add(out=t01[:],
                                     in0=g4[:, tt * NF:tt * NF + 2, :],
                                     in1=g4[:, tt * NF + 2:tt * NF + 4, :])
                z_t = work_p.tile([P, D], bf16, tag="z_t")
                nc.vector.tensor_add(out=z_t[:], in0=t01[:, 0, :],
                                     in1=t01[:, 1, :])
                nc.sync.dma_start(out=z_shard[t * P:t * P + valid, :],
                                  in_=z_t[:valid, :])

        for s, (rbase, nr) in enumerate(SL_ROWS):
            nc.gpsimd.collective_compute(
                "AllGather", mybir.AluOpType.bypass,
                replica_groups=groups,
                ins=[z_shard[rbase:rbase + nr, :]],
                outs=[z_full[s][:]])

        # =================================================================
        # Phases D/E: message passing + MLP layers
        # =================================================================
        def mp_layer(layer):
            tab_full = z_full if layer == 1 else z2_full
            tab_own = z_shard if layer == 1 else z2_shard
            wb_sb = w1b_sb if layer == 1 else w2b_sb
            ba_row, bb_row = (0, 1) if layer == 1 else (2, 3)
            g_row, b_row = (4, 5) if layer == 1 else (6, 7)

            for g in range(n_groups):
                calls = call_cols[g]
                g_lo = calls[0][1]
                g_hi = calls[-1][2]
                Cg = g_hi - g_lo
                if Cg == 0:
                    continue
                # --- metadata ---
                idx_sb = meta_p.tile([128, Cg_max * 8], i16, tag="idx")
                nc.sync.dma_start(
                    out=idx_sb[:, :Cg * 8],
                    in_=idx_d[:, g_lo * 8:g_hi * 8])
                drel = meta_p.tile([P, Cg_max], f16, tag="drel")
                nc.sync.dma_start(out=drel[:, :Cg],
                                  in_=drel_d[:, g_lo:g_hi])
                # --- gather: <=8-chunk calls, one bucket each ---
                G = gath_p.tile([P, Cg_max, D], bf16, tag="G")
                for b, c_lo, c_hi, nv in calls:
                    nb = c_hi - c_lo
                    if nb == 0 or nv == 0:
                        continue
                    inst = nc.gpsimd.dma_gather(
                        out_ap=G[:, c_lo - g_lo:c_hi - g_lo, :],
                        idxs_ap=idx_sb[:, (c_lo - g_lo) * 8:(c_hi - g_lo) * 8],
                        in_ap=tab_full[b][:, :],
                        num_idxs=nb * P, num_idxs_reg=nv, elem_size=D,
                        queue_num=next_q())
                    emit_names.append(inst.ins.name)
                for t in range(g * GSZ, min((g + 1) * GSZ, NT)):
                    tt = t - g * GSZ
                    valid = LAST_VALID if t == NT - 1 else P
                    ranges = tile_ranges[t]
                    ncols = sum(hi - lo for lo, hi in ranges)
                    # --- tile one-hot: S[e, ci, d] = (drel_enc == d+128*tt),
                    # built per bucket range against the shifted iota ---
                    S = sel_p.tile([P, S_max, P], bf16, tag="S")
                    ci0 = 0
                    for lo, hi in ranges:
                        w = hi - lo
                        nc.vector.tensor_tensor(
                            out=S[:, ci0:ci0 + w, :],
                            in0=drel[:, lo - g_lo:hi - g_lo].unsqueeze(2)
                                .to_broadcast([P, w, P]),
                            in1=iota_sb[:, tt, :].unsqueeze(1)
                                .to_broadcast([P, w, P]),
                            op=mybir.AluOpType.is_equal)
                        ci0 += w
                    # --- aggregate: agg[d, :] += S[:, c, d].T @ G[:, c, :] ---
                    agg_ps = ps_agg.tile([P, D], f32, tag="agg")
                    ci = 0
                    for lo, hi in ranges:
                        for c in range(lo - g_lo, hi - g_lo):
                            nc.tensor.matmul(out=agg_ps[:],
                                             lhsT=S[:, ci, :], rhs=G[:, c, :],
                                             start=(ci == 0),
                                             stop=(ci == ncols - 1))
                            ci += 1
                    # --- u = relu(z_own + agg (+ba)) ---
                    zown = work_p.tile([P, D], bf16, tag="zown")
                    if valid < P:
                        nc.vector.memset(zown[:], 0.0)
                    nc.sync.dma_start(out=zown[:valid, :],
                                      in_=tab_own[t * P:t * P + valid, :])
                    u = work_p.tile([P, D], f32, tag="u")
                    nc.vector.tensor_add(out=u[:], in0=agg_ps[:], in1=zown[:])
                    if use_biases:
                        nc.vector.tensor_add(out=u[:], in0=u[:],
                                             in1=bias_sb[ba_row])
                    ur = work_p.tile([P, D], f32, tag="ur")
                    nc.scalar.activation(out=ur[:], in_=u[:], func=AF.Relu)
                    # --- v = u @ wb (+bb) ---
                    uT_ps = ps_tr.tile([P, 2, P], f32, tag="uT_ps")
                    for kk in range(2):
                        nc.tensor.transpose(out=uT_ps[:, kk, :],
                                            in_=ur[:, kk * P:(kk + 1) * P],
                                            identity=ident_sb[:])
                    uT = work_p.tile([P, 2, P], bf16, tag="uT")
                    nc.scalar.activation(out=uT[:], in_=uT_ps[:], func=AF.Copy)
                    v_ps = ps_mm.tile([P, D], f32, tag="v_ps")
                    for kk in range(2):
                        nc.tensor.matmul(out=v_ps[:],
                                         lhsT=uT[:, kk, :], rhs=wb_sb[kk][:],
                                         start=(kk == 0), stop=(kk == 1))
                    r = work_p.tile([P, D], f32, tag="r")
                    if use_biases:
                        nc.vector.tensor_add(out=r[:], in0=v_ps[:],
                                             in1=bias_sb[bb_row])
                        if layer == 1:
                            nc.vector.tensor_scalar_max(out=r[:], in0=r[:],
                                                        scalar1=0.0)
                    else:
                        nc.scalar.activation(
                            out=r[:], in_=v_ps[:],
                            func=AF.Relu if layer == 1 else AF.Copy)
                    # --- LayerNorm ---
                    stats = stat_p.tile([P, 6], f32, tag="stats")
                    nc.vector.bn_stats(out=stats[:], in_=r[:])
                    mv = stat_p.tile([P, 2], f32, tag="mv")
                    nc.vector.bn_aggr(out=mv[:], in_=stats[:])
                    nc.scalar.activation(out=mv[:, 1:2], in_=mv[:, 1:2],
                                         func=AF.Sqrt,
                                         bias=eps_sb[:], scale=1.0)
                    nc.vector.reciprocal(out=mv[:, 1:2], in_=mv[:, 1:2])
                    nm = stat_p.tile([P, 1], f32, tag="nm")
                    nc.vector.tensor_scalar(out=nm[:], in0=mv[:, 0:1],
                                            scalar1=mv[:, 1:2], scalar2=-1.0,
                                            op0=mybir.AluOpType.mult,
                                            op1=mybir.AluOpType.mult)
                    h = work_p.tile([P, D], f32, tag="h")
                    nc.scalar.activation(out=h[:], in_=r[:], func=AF.Identity,
                                         bias=nm[:], scale=mv[:, 1:2])
                    if use_ln_gb:
                        nc.vector.tensor_mul(out=h[:], in0=h[:],
                                             in1=bias_sb[g_row])
                        nc.vector.tensor_add(out=h[:], in0=h[:],
                                             in1=bias_sb[b_row])

                    if layer == 1:
                        # --- z2 = h @ w2a -> z2_shard ---
                        hT_ps = ps_tr.tile([P, 2, P], f32, tag="uT_ps")
                        for kk in range(2):
                            nc.tensor.transpose(out=hT_ps[:, kk, :],
                                                in_=h[:, kk * P:(kk + 1) * P],
                                                identity=ident_sb[:])
                        hT = work_p.tile([P, 2, P], bf16, tag="uT")
                        nc.scalar.activation(out=hT[:], in_=hT_ps[:],
                                             func=AF.Copy)
                        z2_ps = ps_mm.tile([P, D], f32, tag="v_ps")
                        for kk in range(2):
                            nc.tensor.matmul(out=z2_ps[:],
                                             lhsT=hT[:, kk, :],
                                             rhs=w2a_sb[kk][:],
                                             start=(kk == 0), stop=(kk == 1))
                        z2_sb = work_p.tile([P, D], bf16, tag="z2_sb")
                        nc.scalar.activation(out=z2_sb[:], in_=z2_ps[:],
                                             func=AF.Copy)
                        nc.sync.dma_start(
                            out=z2_shard[t * P:t * P + valid, :],
                            in_=z2_sb[:valid, :])
                        if t in sl_of_tile:
                            s = sl_of_tile[t]
                            rbase, nr = SL_ROWS[s]
                            nc.gpsimd.collective_compute(
                                "AllGather", mybir.AluOpType.bypass,
                                replica_groups=groups,
                                ins=[z2_shard[rbase:rbase + nr, :]],
                                outs=[z2_full[s][:]])
                    else:
                        nc.sync.dma_start(out=out_d[t * P:t * P + valid, :],
                                          in_=h[:valid, :])

        mp_layer(1)
        mp_layer(2)

    from concourse.tile_sem_assignment import DMAInst
    final_names = []
    for fn in nc.m.functions:
        for blk in fn.blocks:
            for inst in blk.instructions:
                if (isinstance(inst, DMAInst)
                        and inst.engine == mybir.EngineType.Pool):
                    final_names.append(inst.name)
    emit_idx = {nm: i for i, nm in enumerate(emit_names)}
    qmap = [0] * len(emit_names)
    aligned = True
    for pos, nm in enumerate(final_names):
        qmap[emit_idx[nm]] = pos % 4
        if queue_map is not None and queue_map[emit_idx[nm]] != pos % 4:
            aligned = False
    if compile_now:
        nc.compile()
    return nc, qmap, aligned


def get_program(Cb, call_cols, tile_ranges, **kw):
    key = (tuple(tuple(c) for c in Cb),
           tuple(tuple(c) for cs in call_cols for c in cs),
           tuple(sorted(kw.items())))
    if key not in _PROGRAM_CACHE:
        _, qmap, _ = _build_program(Cb, call_cols, tile_ranges,
                                    compile_now=False, **kw)
        for _ in range(3):
            nc, qmap2, aligned = _build_program(
                Cb, call_cols, tile_ranges, queue_map=qmap,
                compile_now=True, **kw)
            if aligned:
                break
            qmap = qmap2
        assert aligned, "SWDGE queue assignment did not converge"
        _PROGRAM_CACHE[key] = nc
    return _PROGRAM_CACHE[key]


# ---------------------------------------------------------------------------
# Entry point
# ---------------------------------------------------------------------------
def kernel_with_results(x_cat, edge_index, emb0, emb1, emb2, emb3,
                        w1a, b1a, w1b, b1b, w2a, b2a, w2b, b2b,
                        ln1_g, ln1_b, ln2_g, ln2_b, trace=False):
    import ml_dtypes
    from concourse import bass_utils

    Cb, call_cols, tile_ranges, cmeta16, idx_meta, drel_meta = _prep_meta(
        x_cat, edge_index)

    f32 = np.float32
    embT = np.stack([np.ascontiguousarray(np.asarray(e, f32).T)
                     for e in (emb0, emb1, emb2, emb3)])
    w1a = np.ascontiguousarray(np.asarray(w1a, f32))
    w1b = np.ascontiguousarray(np.asarray(w1b, f32))
    w2a = np.ascontiguousarray(np.asarray(w2a, f32))
    w2b = np.ascontiguousarray(np.asarray(w2b, f32))

    biases = [np.asarray(b, f32) for b in (b1a, b1b, b2a, b2b)]
    lngb = [np.asarray(b, f32) for b in (ln1_g, ln1_b, ln2_g, ln2_b)]
    use_biases = any(np.any(b != 0.0) for b in biases)
    use_ln_gb = (np.any(lngb[0] != 1.0) or np.any(lngb[1] != 0.0)
                 or np.any(lngb[2] != 1.0) or np.any(lngb[3] != 0.0))

    iota_row = np.stack([
        np.broadcast_to((np.arange(P) + P * tt).astype(np.float16), (P, P))
        for tt in range(GSZ)]).copy()
    identity = np.eye(P, dtype=f32)

    nc = get_program(Cb, call_cols, tile_ranges, use_biases=use_biases,
                     use_ln_gb=use_ln_gb)

    in_maps = []
    for k in range(NC):
        m = {
            "embT": embT,
            "w1a": w1a, "w1b": w1b, "w2a": w2a, "w2b": w2b,
            "cmeta16": cmeta16[k],
            "idx_meta": idx_meta[k],
            "drel_meta": drel_meta[k],
            "iota_row": iota_row,
            "identity": identity,
        }
        if use_biases or use_ln_gb:
            m["biasrows"] = np.stack(biases + lngb)
        in_maps.append(m)

    res = bass_utils.run_bass_kernel_spmd(nc, in_maps, core_ids=list(range(NC)),
                                          trace=trace)
    out = np.concatenate([r["out"] for r in res.results], axis=0)
    return out.astype(np.float32), res


def kernel(**inputs):
    out, _ = kernel_with_results(**inputs)
    return out
